# revision 3
# baseline (speedup 1.0000x reference)
"""Multi-head causal self-attention (B=2, T=2048, D=1024, H=16, Dh=64) on 8 TRN2 cores.

Sharding: data-parallel over batch (2 groups of 4 cores), tensor-parallel over
heads within a group (4 heads/core). Host sums the 4 partial outputs per batch.

v2 design (per core), driven by the TimelineSim cost model (matmul cost =
out-free-size x cycles/row; fp8e4+DoubleRow = 0.5, bf16 = 1.0 at any N):
  - QKV projections run in fp8 with an exact-to-~0.2% residual scheme:
    x is shipped as x8 + xr8 (fp8 value + fp8 residual), W as hi + lo fp8.
    Per d-chunk: one DoubleRow matmul computes (Whi+Wlo)@x8 (hi/lo stacked in
    the two k-tiles); per chunk-pair one DoubleRow matmul computes Whi@xr8.
    => 0.75 cycles/row instead of 1.0, with bf16-class accuracy.
  - Scores S^T = K^T Q per 128-chunk in bf16 (charge = S elements / 128).
    exp on ACT in chunk-pairs ([128,1024] per instruction) to amortize the
    ~370ns per-instruction SBUF-access overhead; diagonal chunks get
    column-trimmed singles + a Pool-engine triangular mask multiply.
  - PV is FLIPPED: out y[tq=128, dh+1] with se as stationary and v (+ones
    column) as moving => charge 65/chunk instead of 512 (bf16 has no N>=256
    requirement). Accumulation is per (head, tq-block) over tk chunks; causal
    skips diagonal chunks r > b. Softmax sums come from the ones column; DVE
    reciprocal + per-partition tensor_scalar multiply normalizes and casts to
    bf16; a PE transpose (vs shipped identity) restores yT for the output
    projection; DVE drains the transposed tile to SBUF.
  - Output projection in bf16; output DMA'd as bf16 and upcast on host.
  - Engine budget: PE ~93us, ACT (exp) ~79us, DVE (drains+normalize) ~70us,
    Pool (masks) ~25us. PE instructions are emitted manually interleaved
    (S-blocks / PV units / projection units) so PSUM WAR chains never
    head-of-line block the PE queue.  GPSIMD cannot touch PSUM, so all PSUM
    drains are on DVE/ACT.
"""
import sys

import numpy as np
import ml_dtypes

for _p in ("/opt/trn_rl_repo", "/root/.axon_site/_ro/trn_rl_repo"):
    if _p not in sys.path:
        try:
            import concourse  # noqa: F401
            break
        except ImportError:
            sys.path.append(_p)

import concourse.bass as bass  # noqa: E402
import concourse.tile as tile  # noqa: E402
from concourse import bacc, mybir  # noqa: E402
from concourse.bass_utils import run_bass_kernel_spmd  # noqa: E402

P = 128
T = 2048
D = 1024
NH = 4          # heads per core
DH = 64
F = NH * DH     # per-core head features (256)
DC = D // P     # 8 contraction chunks
TJ = T // 512   # 4 tq slices
TC = T // P     # 16 tk chunks
N_CORES = 8
F8 = mybir.dt.float8e4
BF = mybir.dt.bfloat16
F32 = mybir.dt.float32
FR = mybir.dt.float32r
DR = mybir.MatmulPerfMode.DoubleRow
AF = mybir.ActivationFunctionType


def build(dump_debug=False):
    nc = bacc.Bacc("TRN2", target_bir_lowering=False, debug=False, num_devices=N_CORES)
    xd = nc.dram_tensor("xd", [P, DC, 2, T], F8, kind="ExternalInput").ap()
    wq = nc.dram_tensor("wq", [P, DC, 2, F], F8, kind="ExternalInput").ap()
    wk = nc.dram_tensor("wk", [P, DC, 2, F], F8, kind="ExternalInput").ap()
    wv = nc.dram_tensor("wv", [P, DC, 2, F], F8, kind="ExternalInput").ap()
    wo = nc.dram_tensor("wo", [P, 2, D], BF, kind="ExternalInput").ap()
    mask = nc.dram_tensor("mask", [P, 4, 512], BF, kind="ExternalInput").ap()
    ident = nc.dram_tensor("ident", [P, P], BF, kind="ExternalInput").ap()
    out = nc.dram_tensor("out", [T, D], BF, kind="ExternalOutput").ap()
    if dump_debug:
        dbg_q = nc.dram_tensor("dbg_q", [P, 2, T], FR, kind="ExternalOutput").ap()
        dbg_k = nc.dram_tensor("dbg_k", [P, 2, T], FR, kind="ExternalOutput").ap()
        dbg_v = nc.dram_tensor("dbg_v", [P, NH, TC, DH + 1], BF, kind="ExternalOutput").ap()
        dbg_y = nc.dram_tensor("dbg_y", [P, 2, T], BF, kind="ExternalOutput").ap()
        dbg_se = nc.dram_tensor("dbg_se", [P, 2, 512], BF, kind="ExternalOutput").ap()
        dbg_se1 = nc.dram_tensor("dbg_se1", [P, 2, 512], BF, kind="ExternalOutput").ap()
        dbg_sp = nc.dram_tensor("dbg_sp", [P, 2, 512], F32, kind="ExternalOutput").ap()

    # Wq/Wk/Wv are host-scaled by 16 so their fp8 residuals don't underflow
    # e4m3's exponent range; q.k then carries 256x, absorbed into the exp
    # scale. v carries 16x, absorbed in the output-projection drain.
    scale = 1.0 / (np.sqrt(DH) * 256.0)
    OSCALE = 1.0 / 16.0

    with tile.TileContext(nc) as tc:
        with (
            tc.tile_pool(name="weights", bufs=1) as wpool,
            tc.tile_pool(name="persist", bufs=1) as persist,
            tc.tile_pool(name="x", bufs=2) as xpool,
            tc.tile_pool(name="sexp", bufs=1) as sepool,
            tc.tile_pool(name="small", bufs=4) as small,
            tc.tile_pool(name="outsb", bufs=4) as opool,
            tc.tile_pool(name="ps", bufs=1, space="PSUM") as psum,
        ):
            wq_sb = wpool.tile([P, DC, 2, F], F8)
            wk_sb = wpool.tile([P, DC, 2, F], F8)
            wv_sb = wpool.tile([P, DC, 2, F], F8)
            wo_sb = wpool.tile([P, 2, D], BF)
            mask_sb = wpool.tile([P, 4, 512], BF)
            id_sb = wpool.tile([P, P], BF)

            # q/k live in fp32r: the scores matmuls then self-load weights
            # (no 50ns-SEQ Ldweights per matmul, unlike 16-bit dtypes).
            qT_sb = persist.tile([P, 2, T], FR)
            kT_sb = persist.tile([P, 2, T], FR)
            v_sb = persist.tile([P, NH, TC, DH + 1], BF)
            yT_sb = persist.tile([P, 2, T], BF)

            x_tiles = {}

            def load_x(j):
                x_sb = xpool.tile([P, DC, 2, 512], F8, tag="x", name=f"x_{j}")
                nc.sync.dma_start(x_sb[:], xd[:, :, :, 512 * j : 512 * (j + 1)])
                x_tiles[j] = x_sb

            # Startup preload: few big DMAs (each dma_start costs ~625ns of
            # exclusive HWDGE on top of the transfer, so per-chunk interleave
            # makes the first projection DMA-latency-paced).
            x0_sb = xpool.tile([P, DC, 2, 512], F8, tag="x", name="x_0")
            x_tiles[0] = x0_sb
            nc.sync.dma_start(x0_sb[:, 0:4], xd[:, 0:4, :, 0:512])
            nc.sync.dma_start(wq_sb[:], wq)
            nc.sync.dma_start(x0_sb[:, 4:8], xd[:, 4:8, :, 0:512])
            nc.sync.dma_start(wk_sb[:], wk)
            nc.sync.dma_start(wv_sb[:], wv)
            nc.sync.dma_start(mask_sb[:], mask)
            nc.sync.dma_start(id_sb[:], ident)
            load_x(1)
            nc.sync.dma_start(wo_sb[:], wo)

            # ones column of V (softmax sums); gpsimd memset is SBUF-only.
            nc.gpsimd.memset(v_sb[:, :, :, DH : DH + 1], 1.0)

            # --- projection units (fp8 residual DoubleRow) ---
            def proj_qk_unit(j, w_sb, dst, c):
                jsl = slice(512 * j, 512 * (j + 1))
                x_sb = x_tiles[j]
                pt = psum.tile([P, 512], F32, tag="ao", bufs=2, name=f"pqk_{j}_{c}")
                n_i = DC + DC // 2
                k = 0
                for o in range(DC):
                    nc.tensor.matmul(
                        pt[:],
                        w_sb[:, o, :, 128 * c : 128 * (c + 1)],
                        x_sb[:, o, 0, None, :].to_broadcast([P, 2, 512]),
                        start=(k == 0), stop=(k == n_i - 1), perf_mode=DR,
                    )
                    k += 1
                for o in range(0, DC, 2):
                    nc.tensor.matmul(
                        pt[:],
                        w_sb[:, o : o + 2, 0, 128 * c : 128 * (c + 1)],
                        x_sb[:, o : o + 2, 1, :],
                        start=(k == 0), stop=(k == n_i - 1), perf_mode=DR,
                    )
                    k += 1
                nc.vector.tensor_copy(dst[:, c, jsl], pt[:])

            def proj_v_unit(j, i):
                x_sb = x_tiles[j]
                pt = psum.tile([P, 512], F32, tag="ao", bufs=2, name=f"pv_{j}_{i}")
                n_i = DC + DC // 2
                k = 0
                for o in range(DC):
                    nc.tensor.matmul(
                        pt[:, :F],
                        x_sb[:, o, :, 128 * i : 128 * (i + 1)],
                        wv_sb[:, o, 0, None, :].to_broadcast([P, 2, F]),
                        start=(k == 0), stop=(k == n_i - 1), perf_mode=DR,
                    )
                    k += 1
                for o in range(0, DC, 2):
                    nc.tensor.matmul(
                        pt[:, :F],
                        x_sb[:, o : o + 2, 0, 128 * i : 128 * (i + 1)],
                        wv_sb[:, o : o + 2, 1, :],
                        start=(k == 0), stop=(k == n_i - 1), perf_mode=DR,
                    )
                    k += 1
                nc.vector.tensor_copy(
                    v_sb[:, :, 4 * j + i, 0:DH],
                    pt[:, :F].rearrange("p (h d) -> p h d", h=NH),
                )

            def outproj_unit(j, tb, eb):
                while pend_t:
                    flush_transpose()
                pt = psum.tile([P, 512], F32, tag="ao", bufs=2, name=f"o_{j}_{tb}_{eb}")
                for g in range(2):
                    nc.tensor.matmul(
                        pt[:],
                        yT_sb[:, g, 128 * (4 * j + tb) : 128 * (4 * j + tb + 1)],
                        wo_sb[:, g, 512 * eb : 512 * (eb + 1)],
                        start=(g == 0), stop=(g == 1),
                    )
                osb = opool.tile([P, 512], BF, tag="osb", name=f"osb_{j}_{tb}_{eb}")
                if j == TJ - 1:
                    nc.scalar.mul(osb[:], pt[:], OSCALE)
                else:
                    nc.vector.tensor_scalar_mul(osb[:], pt[:], OSCALE)
                nc.sync.dma_start(
                    out[
                        128 * (4 * j + tb) : 128 * (4 * j + tb + 1),
                        512 * eb : 512 * (eb + 1),
                    ],
                    osb[:],
                )

            # --- attention emitters ---
            se_map = {}

            def s_pair(j, h, pp):
                c, tt = h >> 1, h & 1
                hp = DH * tt
                jsl = slice(512 * j, 512 * (j + 1))
                sps = psum.tile([P, 2, 512], F32, tag="sp", bufs=2,
                                name=f"sp_{j}_{h}_{pp}")
                sep = sepool.tile([P, 2, 512], BF, tag="sep", bufs=32,
                                  name=f"sep_{j}_{h}_{pp}")
                for q in range(2):
                    ii = 2 * pp + q
                    nc.tensor.matmul(
                        sps[:, q, :],
                        kT_sb[hp : hp + DH, c, 128 * ii : 128 * (ii + 1)],
                        qT_sb[hp : hp + DH, c, jsl],
                        start=True, stop=True,
                    )
                    se_map[(h, ii)] = (sep, q)
                nc.scalar.activation(sep[:], sps[:], AF.Exp, scale=scale)

            def s_diag2(j, c, r):
                # Both heads of c-pair share the diagonal width, so their two
                # S chunks go in one "sp" pair slot and get ONE exp + ONE
                # (broadcast-mask) multiply.
                ii = 4 * j + r
                col0 = 128 * r
                sdp = psum.tile([P, 2, 512], F32, tag="sp", bufs=2,
                                name=f"sd_{j}_{c}_{r}")
                sed = sepool.tile([P, 2, 512], BF, tag="sep", bufs=32,
                                  name=f"sed_{j}_{c}_{r}")
                # fp32r matmuls with N<256 cost 4 cycles/row; widen r=3 to
                # N=256 (extra columns land in never-read se space).
                mcol0 = min(col0, 512 - 256)
                for tt in range(2):
                    hp = DH * tt
                    nc.tensor.matmul(
                        sdp[:, tt, mcol0:],
                        kT_sb[hp : hp + DH, c, 128 * ii : 128 * (ii + 1)],
                        qT_sb[hp : hp + DH, c, 512 * j + mcol0 : 512 * (j + 1)],
                        start=True, stop=True,
                    )
                    se_map[(2 * c + tt, ii)] = (sed, tt)
                if dump_debug and (j, c, r) == (0, 0, 0):
                    spc = small.tile([P, 2, 512], F32, tag="dbgsp", name="spc")
                    nc.vector.tensor_copy(spc[:], sdp[:])
                    nc.sync.dma_start(dbg_sp, spc[:])
                nc.scalar.activation(
                    sed[:, :, col0:], sdp[:, :, col0:], AF.Exp, scale=scale
                )
                nc.gpsimd.tensor_mul(
                    sed[:, :, col0 : col0 + 128],
                    sed[:, :, col0 : col0 + 128],
                    mask_sb[:, r, None, col0 : col0 + 128].to_broadcast([P, 2, 128]),
                )
                if dump_debug and (j, c, r) == (0, 0, 0):
                    nc.sync.dma_start(dbg_se, sed[:])
                if dump_debug and (j, c, r) == (0, 0, 1):
                    nc.sync.dma_start(dbg_se1, sed[:])

            pend_t = []

            def flush_transpose():
                jj, h, b, yf = pend_t.pop(0)
                c, tt = h >> 1, h & 1
                hp = DH * tt
                pt = psum.tile([P, P], BF, tag="ao", bufs=2, name=f"t_{jj}_{h}_{b}")
                nc.tensor.transpose(pt[hp : hp + DH, :], yf[:], id_sb[:])
                nc.vector.tensor_copy(
                    yT_sb[hp : hp + DH, c, 512 * jj + 128 * b : 512 * jj + 128 * (b + 1)],
                    pt[hp : hp + DH, :],
                )

            def pv_unit(j, h, b):
                yps = psum.tile([P, DH + 1], F32, tag="y", bufs=2,
                                name=f"y_{j}_{h}_{b}")
                chunks = list(range(4 * j)) + [4 * j + r for r in range(b + 1)]
                for idx, ii in enumerate(chunks):
                    sep, half = se_map[(h, ii)]
                    st = (sep[:, half, 128 * b : 128 * (b + 1)]
                          if half is not None
                          else sep[:, 128 * b : 128 * (b + 1)])
                    nc.tensor.matmul(
                        yps[:], st, v_sb[:, h, ii, :],
                        start=(idx == 0), stop=(idx == len(chunks) - 1),
                    )
                rec = small.tile([P, 1], F32, tag="rec", name=f"rec_{j}_{h}_{b}")
                nc.vector.reciprocal(rec[:], yps[:, DH : DH + 1])
                yf = small.tile([P, DH], BF, tag="yf", name=f"yf_{j}_{h}_{b}")
                nc.vector.tensor_scalar_mul(yf[:], yps[:, 0:DH], rec[:])
                pend_t.append((j, h, b, yf))
                if len(pend_t) > 1:
                    flush_transpose()

            # --- schedule ---
            # Filler distribution targets per-slice PE ~= per-slice ACT (exp
            # grows linearly with j, so out-projections are deferred to the
            # later, exp-heavy slices).
            def proj_units(j):
                u = []
                for c in range(2):
                    u.append(lambda c=c: proj_qk_unit(j, wq_sb, qT_sb, c))
                    u.append(lambda c=c: proj_qk_unit(j, wk_sb, kT_sb, c))
                for i in range(4):
                    u.append(lambda i=i: proj_v_unit(j, i))
                return u

            def outproj_units(j):
                return [
                    lambda tb=tb, eb=eb: outproj_unit(j, tb, eb)
                    for tb in range(4)
                    for eb in range(2)
                ]

            p0 = proj_units(0)
            p0[0]()  # q c0
            p0[2]()  # k c0
            p0[1]()  # q c1
            p0[3]()  # k c1
            slice_fillers = {
                0: [p0[4], p0[5], p0[6], p0[7]] + proj_units(1),
                1: proj_units(2),
                2: proj_units(3) + outproj_units(0),
                3: outproj_units(1) + outproj_units(2),
            }

            for j in range(TJ):
                fillers = slice_fillers[j]
                if j + 2 < TJ:
                    load_x(j + 2)

                pv_queue = []
                tick = [0]

                def pump():
                    # Alternate filler/PV so deadline-ordered fillers (q/k
                    # c1 before head 2 of slice 0; projections feeding the
                    # next slice) drain even while PV units are queued.
                    tick[0] ^= 1
                    if fillers and (tick[0] or not pv_queue):
                        fillers.pop(0)()
                    elif pv_queue:
                        h, b = pv_queue.pop(0)
                        pv_unit(j, h, b)

                for c in range(2):
                    for tt in range(2):
                        for pp in range(2 * j):
                            s_pair(j, 2 * c + tt, pp)
                            pump()
                    for r in range(4):
                        s_diag2(j, c, r)
                        pump()
                    pv_queue.extend((2 * c + tt, b) for b in range(4) for tt in range(2))

                if j < TJ - 1:
                    while pv_queue or fillers:
                        pump()
                    while pend_t:
                        flush_transpose()
                else:
                    # Tail: weave the last slice's output projection in as
                    # each tq-block's final transpose lands.
                    while pv_queue:
                        h, b = pv_queue.pop(0)
                        pv_unit(j, h, b)
                        if fillers:
                            fillers.pop(0)()
                        if h == NH - 1 and b > 0:
                            while pend_t and pend_t[0][2] < b:
                                flush_transpose()
                            outproj_unit(j, b - 1, 0)
                            outproj_unit(j, b - 1, 1)
                    while pend_t:
                        flush_transpose()
                    while fillers:
                        fillers.pop(0)()
                    outproj_unit(j, 3, 0)
                    outproj_unit(j, 3, 1)

            if dump_debug:
                nc.sync.dma_start(dbg_q, qT_sb[:])
                nc.sync.dma_start(dbg_k, kT_sb[:])
                nc.sync.dma_start(dbg_v, v_sb[:])
                nc.sync.dma_start(dbg_y, yT_sb[:])
    # move_matmul_waits_to_ldweights moves a matmul's sem waits to "the most
    # recent Ldweights". fp32r matmuls have no Ldweights of their own, so in
    # this mixed fp8/fp32r kernel the pass relocates their RAW waits onto an
    # unrelated earlier fp8 Ldweights, dropping the ordering (observed: the
    # first scores matmul reading q/k before the projection drains). Disable.
    nc.move_matmul_waits_to_ldweights = lambda: None
    nc.compile()
    return nc


def make_mask() -> np.ndarray:
    q = np.arange(512)[None, None, :]
    p = np.arange(P)[:, None, None]
    r = np.arange(4)[None, :, None]
    return (q >= 128 * r + p).astype(ml_dtypes.bfloat16)


def _hilo(a: np.ndarray):
    hi = a.astype(ml_dtypes.float8_e4m3)
    lo = (a - hi.astype(np.float32)).astype(ml_dtypes.float8_e4m3)
    return hi, lo


def _chunked_hilo(a: np.ndarray, width: int, pre_scale: float = 1.0):
    """[D, width] f32 -> [P, DC, 2, width] fp8 (hi/lo interleaved)."""
    hi, lo = _hilo(a * pre_scale)
    s = np.stack([hi.reshape(DC, P, width), lo.reshape(DC, P, width)], axis=2)
    return np.ascontiguousarray(s.transpose(1, 0, 2, 3))


def shard_inputs(x, Wqkv, Wout):
    mask = make_mask()
    identity = np.eye(P).astype(ml_dtypes.bfloat16)
    in_maps = []
    for c in range(N_CORES):
        b, g = c // 4, c % 4
        sl = slice(F * g, F * (g + 1))
        xT = np.ascontiguousarray(x[b].T)  # [D, T]
        woT = np.ascontiguousarray(Wout[:, sl].T).astype(ml_dtypes.bfloat16)  # [F, D]
        in_maps.append(
            {
                "xd": _chunked_hilo(xT, T),
                "wq": _chunked_hilo(np.ascontiguousarray(Wqkv[sl, :].T), F, 16.0),
                "wk": _chunked_hilo(np.ascontiguousarray(Wqkv[D:][sl, :].T), F, 16.0),
                "wv": _chunked_hilo(np.ascontiguousarray(Wqkv[2 * D:][sl, :].T), F, 16.0),
                "wo": np.ascontiguousarray(woT.reshape(2, P, D).transpose(1, 0, 2)),
                "mask": mask,
                "ident": identity,
            }
        )
    return in_maps


_NC_CACHE = None


def kernel(x, Wqkv, Wout):
    global _NC_CACHE
    x = np.asarray(x, dtype=np.float32)
    Wqkv = np.asarray(Wqkv, dtype=np.float32)
    Wout = np.asarray(Wout, dtype=np.float32)
    if _NC_CACHE is None:
        _NC_CACHE = build()
    nc = _NC_CACHE
    in_maps = shard_inputs(x, Wqkv, Wout)
    res = run_bass_kernel_spmd(nc, in_maps, core_ids=list(range(N_CORES)))
    outs = [res.results[c]["out"].astype(np.float32) for c in range(N_CORES)]
    return np.stack(
        [outs[0] + outs[1] + outs[2] + outs[3], outs[4] + outs[5] + outs[6] + outs[7]]
    )


# revision 4
# speedup vs baseline: 1.0290x; 1.0290x over previous
"""Multi-head causal self-attention (B=2, T=2048, D=1024, H=16, Dh=64) on 8 TRN2 cores.

Sharding: data-parallel over batch (2 groups of 4 cores), tensor-parallel over
heads within a group (4 heads/core). Host sums the 4 partial outputs per batch.

v2 design (per core), driven by the TimelineSim cost model (matmul cost =
out-free-size x cycles/row; fp8e4+DoubleRow = 0.5, bf16 = 1.0 at any N):
  - QKV projections run in fp8 with an exact-to-~0.2% residual scheme:
    x is shipped as x8 + xr8 (fp8 value + fp8 residual), W as hi + lo fp8.
    Per d-chunk: one DoubleRow matmul computes (Whi+Wlo)@x8 (hi/lo stacked in
    the two k-tiles); per chunk-pair one DoubleRow matmul computes Whi@xr8.
    => 0.75 cycles/row instead of 1.0, with bf16-class accuracy.
  - Scores S^T = K^T Q per 128-chunk in bf16 (charge = S elements / 128).
    exp on ACT in chunk-pairs ([128,1024] per instruction) to amortize the
    ~370ns per-instruction SBUF-access overhead; diagonal chunks get
    column-trimmed singles + a Pool-engine triangular mask multiply.
  - PV is FLIPPED: out y[tq=128, dh+1] with se as stationary and v (+ones
    column) as moving => charge 65/chunk instead of 512 (bf16 has no N>=256
    requirement). Accumulation is per (head, tq-block) over tk chunks; causal
    skips diagonal chunks r > b. Softmax sums come from the ones column; DVE
    reciprocal + per-partition tensor_scalar multiply normalizes and casts to
    bf16; a PE transpose (vs shipped identity) restores yT for the output
    projection; DVE drains the transposed tile to SBUF.
  - Output projection in bf16; output DMA'd as bf16 and upcast on host.
  - Engine budget: PE ~93us, ACT (exp) ~79us, DVE (drains+normalize) ~70us,
    Pool (masks) ~25us. PE instructions are emitted manually interleaved
    (S-blocks / PV units / projection units) so PSUM WAR chains never
    head-of-line block the PE queue.  GPSIMD cannot touch PSUM, so all PSUM
    drains are on DVE/ACT.
"""
import sys

import numpy as np
import ml_dtypes

for _p in ("/opt/trn_rl_repo", "/root/.axon_site/_ro/trn_rl_repo"):
    if _p not in sys.path:
        try:
            import concourse  # noqa: F401
            break
        except ImportError:
            sys.path.append(_p)

import concourse.bass as bass  # noqa: E402
import concourse.tile as tile  # noqa: E402
from concourse import bacc, mybir  # noqa: E402
from concourse.bass_utils import run_bass_kernel_spmd  # noqa: E402

P = 128
T = 2048
D = 1024
NH = 4          # heads per core
DH = 64
F = NH * DH     # per-core head features (256)
DC = D // P     # 8 contraction chunks
TJ = T // 512   # 4 tq slices
TC = T // P     # 16 tk chunks
N_CORES = 8
F8 = mybir.dt.float8e4
BF = mybir.dt.bfloat16
F32 = mybir.dt.float32
FR = mybir.dt.float32r
DR = mybir.MatmulPerfMode.DoubleRow
AF = mybir.ActivationFunctionType


def build(dump_debug=False):
    nc = bacc.Bacc("TRN2", target_bir_lowering=False, debug=False, num_devices=N_CORES)
    xd = nc.dram_tensor("xd", [P, DC, 2, T], F8, kind="ExternalInput").ap()
    wq = nc.dram_tensor("wq", [P, DC, 2, F], F8, kind="ExternalInput").ap()
    wk = nc.dram_tensor("wk", [P, DC, 2, F], F8, kind="ExternalInput").ap()
    wv = nc.dram_tensor("wv", [P, DC, 2, F], F8, kind="ExternalInput").ap()
    wo = nc.dram_tensor("wo", [P, 2, D], BF, kind="ExternalInput").ap()
    mask = nc.dram_tensor("mask", [P, 4, 512], BF, kind="ExternalInput").ap()
    ident = nc.dram_tensor("ident", [P, P], BF, kind="ExternalInput").ap()
    out = nc.dram_tensor("out", [T, D], BF, kind="ExternalOutput").ap()
    if dump_debug:
        dbg_q = nc.dram_tensor("dbg_q", [P, 2, T], FR, kind="ExternalOutput").ap()
        dbg_k = nc.dram_tensor("dbg_k", [P, 2, T], FR, kind="ExternalOutput").ap()
        dbg_v = nc.dram_tensor("dbg_v", [P, NH, TC, DH + 1], BF, kind="ExternalOutput").ap()
        dbg_y = nc.dram_tensor("dbg_y", [P, 2, T], BF, kind="ExternalOutput").ap()
        dbg_se = nc.dram_tensor("dbg_se", [P, 2, 512], BF, kind="ExternalOutput").ap()
        dbg_se1 = nc.dram_tensor("dbg_se1", [P, 2, 512], BF, kind="ExternalOutput").ap()
        dbg_sp = nc.dram_tensor("dbg_sp", [P, 2, 512], F32, kind="ExternalOutput").ap()

    # Wq/Wk/Wv are host-scaled by 16 so their fp8 residuals don't underflow
    # e4m3's exponent range; q.k then carries 256x, absorbed into the exp
    # scale. v carries 16x, absorbed in the output-projection drain.
    scale = 1.0 / (np.sqrt(DH) * 256.0)
    OSCALE = 1.0 / 16.0

    with tile.TileContext(nc) as tc:
        with (
            tc.tile_pool(name="weights", bufs=1) as wpool,
            tc.tile_pool(name="persist", bufs=1) as persist,
            tc.tile_pool(name="x", bufs=2) as xpool,
            tc.tile_pool(name="sexp", bufs=1) as sepool,
            tc.tile_pool(name="small", bufs=4) as small,
            tc.tile_pool(name="outsb", bufs=4) as opool,
            tc.tile_pool(name="ps", bufs=1, space="PSUM") as psum,
        ):
            wq_sb = wpool.tile([P, DC, 2, F], F8)
            wk_sb = wpool.tile([P, DC, 2, F], F8)
            wv_sb = wpool.tile([P, DC, 2, F], F8)
            wo_sb = wpool.tile([P, 2, D], BF)
            mask_sb = wpool.tile([P, 4, 512], BF)
            id_sb = wpool.tile([P, P], BF)

            # q/k live in fp32r: the scores matmuls then self-load weights
            # (no 50ns-SEQ Ldweights per matmul, unlike 16-bit dtypes).
            qT_sb = persist.tile([P, 2, T], FR)
            kT_sb = persist.tile([P, 2, T], FR)
            v_sb = persist.tile([P, NH, TC, DH + 1], BF)
            yT_sb = persist.tile([P, 2, T], BF)

            x_tiles = {}

            def load_x(j):
                x_sb = xpool.tile([P, DC, 2, 512], F8, tag="x", name=f"x_{j}")
                nc.sync.dma_start(x_sb[:], xd[:, :, :, 512 * j : 512 * (j + 1)])
                x_tiles[j] = x_sb

            # Startup preload: few big DMAs (each dma_start costs ~625ns of
            # exclusive HWDGE on top of the transfer, so per-chunk interleave
            # makes the first projection DMA-latency-paced).
            x0_sb = xpool.tile([P, DC, 2, 512], F8, tag="x", name="x_0")
            x_tiles[0] = x0_sb
            nc.sync.dma_start(x0_sb[:, 0:4], xd[:, 0:4, :, 0:512])
            nc.sync.dma_start(wq_sb[:], wq)
            nc.sync.dma_start(x0_sb[:, 4:8], xd[:, 4:8, :, 0:512])
            nc.sync.dma_start(wk_sb[:], wk)
            nc.sync.dma_start(wv_sb[:], wv)
            nc.sync.dma_start(mask_sb[:], mask)
            nc.sync.dma_start(id_sb[:], ident)
            load_x(1)
            nc.sync.dma_start(wo_sb[:], wo)

            # ones column of V (softmax sums); gpsimd memset is SBUF-only.
            nc.gpsimd.memset(v_sb[:, :, :, DH : DH + 1], 1.0)

            # --- projection units (fp8 residual DoubleRow) ---
            def proj_qk_unit(j, w_sb, dst, c):
                jsl = slice(512 * j, 512 * (j + 1))
                x_sb = x_tiles[j]
                pt = psum.tile([P, 512], F32, tag="ao", bufs=2, name=f"pqk_{j}_{c}")
                n_i = DC + DC // 2
                k = 0
                for o in range(DC):
                    nc.tensor.matmul(
                        pt[:],
                        w_sb[:, o, :, 128 * c : 128 * (c + 1)],
                        x_sb[:, o, 0, None, :].to_broadcast([P, 2, 512]),
                        start=(k == 0), stop=(k == n_i - 1), perf_mode=DR,
                    )
                    k += 1
                for o in range(0, DC, 2):
                    nc.tensor.matmul(
                        pt[:],
                        w_sb[:, o : o + 2, 0, 128 * c : 128 * (c + 1)],
                        x_sb[:, o : o + 2, 1, :],
                        start=(k == 0), stop=(k == n_i - 1), perf_mode=DR,
                    )
                    k += 1
                nc.vector.tensor_copy(dst[:, c, jsl], pt[:])

            def proj_v_unit(j, i):
                x_sb = x_tiles[j]
                pt = psum.tile([P, 512], F32, tag="ao", bufs=2, name=f"pv_{j}_{i}")
                n_i = DC + DC // 2
                k = 0
                for o in range(DC):
                    nc.tensor.matmul(
                        pt[:, :F],
                        x_sb[:, o, :, 128 * i : 128 * (i + 1)],
                        wv_sb[:, o, 0, None, :].to_broadcast([P, 2, F]),
                        start=(k == 0), stop=(k == n_i - 1), perf_mode=DR,
                    )
                    k += 1
                for o in range(0, DC, 2):
                    nc.tensor.matmul(
                        pt[:, :F],
                        x_sb[:, o : o + 2, 0, 128 * i : 128 * (i + 1)],
                        wv_sb[:, o : o + 2, 1, :],
                        start=(k == 0), stop=(k == n_i - 1), perf_mode=DR,
                    )
                    k += 1
                nc.vector.tensor_copy(
                    v_sb[:, :, 4 * j + i, 0:DH],
                    pt[:, :F].rearrange("p (h d) -> p h d", h=NH),
                )

            def outproj_unit(j, tb, eb):
                while pend_t:
                    flush_transpose()
                pt = psum.tile([P, 512], F32, tag="ao", bufs=2, name=f"o_{j}_{tb}_{eb}")
                for g in range(2):
                    nc.tensor.matmul(
                        pt[:],
                        yT_sb[:, g, 128 * (4 * j + tb) : 128 * (4 * j + tb + 1)],
                        wo_sb[:, g, 512 * eb : 512 * (eb + 1)],
                        start=(g == 0), stop=(g == 1),
                    )
                osb = opool.tile([P, 512], BF, tag="osb", name=f"osb_{j}_{tb}_{eb}")
                if j == TJ - 1:
                    nc.scalar.mul(osb[:], pt[:], OSCALE)
                else:
                    nc.vector.tensor_scalar_mul(osb[:], pt[:], OSCALE)
                nc.sync.dma_start(
                    out[
                        128 * (4 * j + tb) : 128 * (4 * j + tb + 1),
                        512 * eb : 512 * (eb + 1),
                    ],
                    osb[:],
                )

            # --- attention emitters ---
            se_map = {}

            def s_pair(j, h, pp):
                c, tt = h >> 1, h & 1
                hp = DH * tt
                jsl = slice(512 * j, 512 * (j + 1))
                sps = psum.tile([P, 2, 512], F32, tag="sp", bufs=2,
                                name=f"sp_{j}_{h}_{pp}")
                sep = sepool.tile([P, 2, 512], BF, tag="sep", bufs=32,
                                  name=f"sep_{j}_{h}_{pp}")
                for q in range(2):
                    ii = 2 * pp + q
                    nc.tensor.matmul(
                        sps[:, q, :],
                        kT_sb[hp : hp + DH, c, 128 * ii : 128 * (ii + 1)],
                        qT_sb[hp : hp + DH, c, jsl],
                        start=True, stop=True,
                    )
                    se_map[(j, h, ii)] = (sep, q)
                nc.scalar.activation(sep[:], sps[:], AF.Exp, scale=scale)

            def s_diag2(j, c, r):
                # Both heads of c-pair share the diagonal width, so their two
                # S chunks go in one "sp" pair slot and get ONE exp + ONE
                # (broadcast-mask) multiply.
                ii = 4 * j + r
                col0 = 128 * r
                sdp = psum.tile([P, 2, 512], F32, tag="sp", bufs=2,
                                name=f"sd_{j}_{c}_{r}")
                sed = sepool.tile([P, 2, 512], BF, tag="sep", bufs=32,
                                  name=f"sed_{j}_{c}_{r}")
                # fp32r matmuls with N<256 cost 4 cycles/row; widen r=3 to
                # N=256 (extra columns land in never-read se space).
                mcol0 = min(col0, 512 - 256)
                for tt in range(2):
                    hp = DH * tt
                    nc.tensor.matmul(
                        sdp[:, tt, mcol0:],
                        kT_sb[hp : hp + DH, c, 128 * ii : 128 * (ii + 1)],
                        qT_sb[hp : hp + DH, c, 512 * j + mcol0 : 512 * (j + 1)],
                        start=True, stop=True,
                    )
                    se_map[(j, 2 * c + tt, ii)] = (sed, tt)
                if dump_debug and (j, c, r) == (0, 0, 0):
                    spc = small.tile([P, 2, 512], F32, tag="dbgsp", name="spc")
                    nc.vector.tensor_copy(spc[:], sdp[:])
                    nc.sync.dma_start(dbg_sp, spc[:])
                nc.scalar.activation(
                    sed[:, :, col0:], sdp[:, :, col0:], AF.Exp, scale=scale
                )
                nc.gpsimd.tensor_mul(
                    sed[:, :, col0 : col0 + 128],
                    sed[:, :, col0 : col0 + 128],
                    mask_sb[:, r, None, col0 : col0 + 128].to_broadcast([P, 2, 128]),
                )
                if dump_debug and (j, c, r) == (0, 0, 0):
                    nc.sync.dma_start(dbg_se, sed[:])
                if dump_debug and (j, c, r) == (0, 0, 1):
                    nc.sync.dma_start(dbg_se1, sed[:])

            pend_t = []

            def flush_transpose():
                jj, h, b, yf = pend_t.pop(0)
                c, tt = h >> 1, h & 1
                hp = DH * tt
                pt = psum.tile([P, P], BF, tag="ao", bufs=2, name=f"t_{jj}_{h}_{b}")
                nc.tensor.transpose(pt[hp : hp + DH, :], yf[:], id_sb[:])
                nc.vector.tensor_copy(
                    yT_sb[hp : hp + DH, c, 512 * jj + 128 * b : 512 * jj + 128 * (b + 1)],
                    pt[hp : hp + DH, :],
                )

            def pv_unit(j, h, b):
                yps = psum.tile([P, DH + 1], F32, tag="y", bufs=2,
                                name=f"y_{j}_{h}_{b}")
                chunks = list(range(4 * j)) + [4 * j + r for r in range(b + 1)]
                for idx, ii in enumerate(chunks):
                    sep, half = se_map[(j, h, ii)]
                    st = (sep[:, half, 128 * b : 128 * (b + 1)]
                          if half is not None
                          else sep[:, 128 * b : 128 * (b + 1)])
                    nc.tensor.matmul(
                        yps[:], st, v_sb[:, h, ii, :],
                        start=(idx == 0), stop=(idx == len(chunks) - 1),
                    )
                rec = small.tile([P, 1], F32, tag="rec", name=f"rec_{j}_{h}_{b}")
                nc.vector.reciprocal(rec[:], yps[:, DH : DH + 1])
                yf = small.tile([P, DH], BF, tag="yf", name=f"yf_{j}_{h}_{b}")
                nc.vector.tensor_scalar_mul(yf[:], yps[:, 0:DH], rec[:])
                pend_t.append((j, h, b, yf))
                if len(pend_t) > 1:
                    flush_transpose()

            # --- schedule ---
            # Filler distribution targets per-slice PE ~= per-slice ACT (exp
            # grows linearly with j, so out-projections are deferred to the
            # later, exp-heavy slices).
            def proj_units(j):
                u = []
                for c in range(2):
                    u.append(lambda c=c: proj_qk_unit(j, wq_sb, qT_sb, c))
                    u.append(lambda c=c: proj_qk_unit(j, wk_sb, kT_sb, c))
                for i in range(4):
                    u.append(lambda i=i: proj_v_unit(j, i))
                return u

            def outproj_units(j):
                return [
                    lambda tb=tb, eb=eb: outproj_unit(j, tb, eb)
                    for tb in range(4)
                    for eb in range(2)
                ]

            p0 = proj_units(0)
            p0[0]()  # q c0
            p0[2]()  # k c0
            p0[1]()  # q c1
            p0[3]()  # k c1
            p3 = proj_units(3)
            slice_fillers = {
                0: [p0[4], p0[5], p0[6], p0[7]] + proj_units(1),
                1: proj_units(2),
                2: [p3[0], p3[2], p3[4], p3[5]] + outproj_units(0),
                3: [p3[1], p3[3], p3[6], p3[7]] + outproj_units(1) + outproj_units(2)[:4],
            }
            tail_reserve = outproj_units(2)[4:]

            pv_queue = []
            for j in range(TJ):
                fillers = slice_fillers[j]
                if j + 2 < TJ:
                    load_x(j + 2)

                tick = [0]

                def pump():
                    # Alternate filler/PV so deadline-ordered fillers (q/k
                    # c1 before head 2 of slice 0; projections feeding the
                    # next slice) drain even while PV units are queued.
                    tick[0] ^= 1
                    if fillers and (tick[0] or not pv_queue):
                        fillers.pop(0)()
                    elif pv_queue:
                        jj, h, b = pv_queue.pop(0)
                        pv_unit(jj, h, b)

                for c in range(2):
                    if c == 1:
                        # Carried PV units of older slices must emit before
                        # this slice's c1 S-tiles rotate into their se slots
                        # (sep pool holds 32 tiles; reads emitted after the
                        # overwriting write would see the new data).
                        while pv_queue and pv_queue[0][0] < j:
                            jj, h, b = pv_queue.pop(0)
                            pv_unit(jj, h, b)
                    for tt in range(2):
                        for pp in range(2 * j):
                            s_pair(j, 2 * c + tt, pp)
                            pump()
                    for r in range(4):
                        s_diag2(j, c, r)
                        pump()
                    pv_queue.extend((j, 2 * c + tt, b) for b in range(4) for tt in range(2))

                if j < TJ - 1:
                    while fillers:
                        pump()
                else:
                    # Tail: weave the last slice's output projection in as
                    # each tq-block's final transpose lands.
                    fillers.extend(tail_reserve)
                    while pv_queue:
                        jj, h, b = pv_queue.pop(0)
                        pv_unit(jj, h, b)
                        if fillers:
                            fillers.pop(0)()
                        if jj == TJ - 1 and h == NH - 1 and b > 0:
                            while pend_t and pend_t[0][2] < b:
                                flush_transpose()
                            outproj_unit(j, b - 1, 0)
                            outproj_unit(j, b - 1, 1)
                    while pend_t:
                        flush_transpose()
                    while fillers:
                        fillers.pop(0)()
                    outproj_unit(j, 3, 0)
                    outproj_unit(j, 3, 1)

            if dump_debug:
                nc.sync.dma_start(dbg_q, qT_sb[:])
                nc.sync.dma_start(dbg_k, kT_sb[:])
                nc.sync.dma_start(dbg_v, v_sb[:])
                nc.sync.dma_start(dbg_y, yT_sb[:])
    # move_matmul_waits_to_ldweights moves a matmul's sem waits to "the most
    # recent Ldweights". fp32r matmuls have no Ldweights of their own, so in
    # this mixed fp8/fp32r kernel the pass relocates their RAW waits onto an
    # unrelated earlier fp8 Ldweights, dropping the ordering (observed: the
    # first scores matmul reading q/k before the projection drains). Disable.
    nc.move_matmul_waits_to_ldweights = lambda: None
    nc.compile()
    return nc


def make_mask() -> np.ndarray:
    q = np.arange(512)[None, None, :]
    p = np.arange(P)[:, None, None]
    r = np.arange(4)[None, :, None]
    return (q >= 128 * r + p).astype(ml_dtypes.bfloat16)


def _hilo(a: np.ndarray):
    hi = a.astype(ml_dtypes.float8_e4m3)
    lo = (a - hi.astype(np.float32)).astype(ml_dtypes.float8_e4m3)
    return hi, lo


def _chunked_hilo(a: np.ndarray, width: int, pre_scale: float = 1.0):
    """[D, width] f32 -> [P, DC, 2, width] fp8 (hi/lo interleaved)."""
    hi, lo = _hilo(a * pre_scale)
    s = np.stack([hi.reshape(DC, P, width), lo.reshape(DC, P, width)], axis=2)
    return np.ascontiguousarray(s.transpose(1, 0, 2, 3))


def shard_inputs(x, Wqkv, Wout):
    mask = make_mask()
    identity = np.eye(P).astype(ml_dtypes.bfloat16)
    in_maps = []
    for c in range(N_CORES):
        b, g = c // 4, c % 4
        sl = slice(F * g, F * (g + 1))
        xT = np.ascontiguousarray(x[b].T)  # [D, T]
        woT = np.ascontiguousarray(Wout[:, sl].T).astype(ml_dtypes.bfloat16)  # [F, D]
        in_maps.append(
            {
                "xd": _chunked_hilo(xT, T),
                "wq": _chunked_hilo(np.ascontiguousarray(Wqkv[sl, :].T), F, 16.0),
                "wk": _chunked_hilo(np.ascontiguousarray(Wqkv[D:][sl, :].T), F, 16.0),
                "wv": _chunked_hilo(np.ascontiguousarray(Wqkv[2 * D:][sl, :].T), F, 16.0),
                "wo": np.ascontiguousarray(woT.reshape(2, P, D).transpose(1, 0, 2)),
                "mask": mask,
                "ident": identity,
            }
        )
    return in_maps


_NC_CACHE = None


def kernel(x, Wqkv, Wout):
    global _NC_CACHE
    x = np.asarray(x, dtype=np.float32)
    Wqkv = np.asarray(Wqkv, dtype=np.float32)
    Wout = np.asarray(Wout, dtype=np.float32)
    if _NC_CACHE is None:
        _NC_CACHE = build()
    nc = _NC_CACHE
    in_maps = shard_inputs(x, Wqkv, Wout)
    res = run_bass_kernel_spmd(nc, in_maps, core_ids=list(range(N_CORES)))
    outs = [res.results[c]["out"].astype(np.float32) for c in range(N_CORES)]
    return np.stack(
        [outs[0] + outs[1] + outs[2] + outs[3], outs[4] + outs[5] + outs[6] + outs[7]]
    )


# revision 5
# speedup vs baseline: 1.0372x; 1.0080x over previous
"""Multi-head causal self-attention (B=2, T=2048, D=1024, H=16, Dh=64) on 8 TRN2 cores.

Sharding: data-parallel over batch (2 groups of 4 cores), tensor-parallel over
heads within a group (4 heads/core). Host sums the 4 partial outputs per batch.

v2 design (per core), driven by the TimelineSim cost model (matmul cost =
out-free-size x cycles/row; fp8e4+DoubleRow = 0.5, bf16 = 1.0 at any N):
  - QKV projections run in fp8 with an exact-to-~0.2% residual scheme:
    x is shipped as x8 + xr8 (fp8 value + fp8 residual), W as hi + lo fp8.
    Per d-chunk: one DoubleRow matmul computes (Whi+Wlo)@x8 (hi/lo stacked in
    the two k-tiles); per chunk-pair one DoubleRow matmul computes Whi@xr8.
    => 0.75 cycles/row instead of 1.0, with bf16-class accuracy.
  - Scores S^T = K^T Q per 128-chunk in bf16 (charge = S elements / 128).
    exp on ACT in chunk-pairs ([128,1024] per instruction) to amortize the
    ~370ns per-instruction SBUF-access overhead; diagonal chunks get
    column-trimmed singles + a Pool-engine triangular mask multiply.
  - PV is FLIPPED: out y[tq=128, dh+1] with se as stationary and v (+ones
    column) as moving => charge 65/chunk instead of 512 (bf16 has no N>=256
    requirement). Accumulation is per (head, tq-block) over tk chunks; causal
    skips diagonal chunks r > b. Softmax sums come from the ones column; DVE
    reciprocal + per-partition tensor_scalar multiply normalizes and casts to
    bf16; a PE transpose (vs shipped identity) restores yT for the output
    projection; DVE drains the transposed tile to SBUF.
  - Output projection in bf16; output DMA'd as bf16 and upcast on host.
  - Engine budget: PE ~93us, ACT (exp) ~79us, DVE (drains+normalize) ~70us,
    Pool (masks) ~25us. PE instructions are emitted manually interleaved
    (S-blocks / PV units / projection units) so PSUM WAR chains never
    head-of-line block the PE queue.  GPSIMD cannot touch PSUM, so all PSUM
    drains are on DVE/ACT.
"""
import sys

import numpy as np
import ml_dtypes

for _p in ("/opt/trn_rl_repo", "/root/.axon_site/_ro/trn_rl_repo"):
    if _p not in sys.path:
        try:
            import concourse  # noqa: F401
            break
        except ImportError:
            sys.path.append(_p)

import concourse.bass as bass  # noqa: E402
import concourse.tile as tile  # noqa: E402
from concourse import bacc, mybir  # noqa: E402
from concourse.bass_utils import run_bass_kernel_spmd  # noqa: E402

P = 128
T = 2048
D = 1024
NH = 4          # heads per core
DH = 64
F = NH * DH     # per-core head features (256)
DC = D // P     # 8 contraction chunks
TJ = T // 512   # 4 tq slices
TC = T // P     # 16 tk chunks
N_CORES = 8
F8 = mybir.dt.float8e4
BF = mybir.dt.bfloat16
F32 = mybir.dt.float32
FR = mybir.dt.float32r
DR = mybir.MatmulPerfMode.DoubleRow
AF = mybir.ActivationFunctionType


def build(dump_debug=False):
    nc = bacc.Bacc("TRN2", target_bir_lowering=False, debug=False, num_devices=N_CORES)
    xd = nc.dram_tensor("xd", [P, DC, 2, T], F8, kind="ExternalInput").ap()
    wq = nc.dram_tensor("wq", [P, DC, 2, F], F8, kind="ExternalInput").ap()
    wk = nc.dram_tensor("wk", [P, DC, 2, F], F8, kind="ExternalInput").ap()
    wv = nc.dram_tensor("wv", [P, DC, 2, F], F8, kind="ExternalInput").ap()
    wo = nc.dram_tensor("wo", [P, 2, D], BF, kind="ExternalInput").ap()
    mask = nc.dram_tensor("mask", [P, 4, 512], BF, kind="ExternalInput").ap()
    ident = nc.dram_tensor("ident", [P, P], BF, kind="ExternalInput").ap()
    out = nc.dram_tensor("out", [T, D], BF, kind="ExternalOutput").ap()
    if dump_debug:
        dbg_q = nc.dram_tensor("dbg_q", [P, 2, T], FR, kind="ExternalOutput").ap()
        dbg_k = nc.dram_tensor("dbg_k", [P, 2, T], FR, kind="ExternalOutput").ap()
        dbg_v = nc.dram_tensor("dbg_v", [P, NH, TC, DH + 1], BF, kind="ExternalOutput").ap()
        dbg_y = nc.dram_tensor("dbg_y", [P, 2, T], BF, kind="ExternalOutput").ap()
        dbg_se = nc.dram_tensor("dbg_se", [P, 2, 512], BF, kind="ExternalOutput").ap()
        dbg_se1 = nc.dram_tensor("dbg_se1", [P, 2, 512], BF, kind="ExternalOutput").ap()
        dbg_sp = nc.dram_tensor("dbg_sp", [P, 2, 512], F32, kind="ExternalOutput").ap()

    # Wq/Wk/Wv are host-scaled by 16 so their fp8 residuals don't underflow
    # e4m3's exponent range; q.k then carries 256x, absorbed into the exp
    # scale. v carries 16x, absorbed in the output-projection drain.
    scale = 1.0 / (np.sqrt(DH) * 256.0)
    OSCALE = 1.0 / 16.0

    with tile.TileContext(nc) as tc:
        with (
            tc.tile_pool(name="weights", bufs=1) as wpool,
            tc.tile_pool(name="persist", bufs=1) as persist,
            tc.tile_pool(name="x", bufs=2) as xpool,
            tc.tile_pool(name="sexp", bufs=1) as sepool,
            tc.tile_pool(name="small", bufs=4) as small,
            tc.tile_pool(name="outsb", bufs=4) as opool,
            tc.tile_pool(name="ps", bufs=1, space="PSUM") as psum,
        ):
            wq_sb = wpool.tile([P, DC, 2, F], F8)
            wk_sb = wpool.tile([P, DC, 2, F], F8)
            wv_sb = wpool.tile([P, DC, 2, F], F8)
            wo_sb = wpool.tile([P, 2, D], BF)
            mask_sb = wpool.tile([P, 4, 512], BF)
            id_sb = wpool.tile([P, P], BF)

            # q/k live in fp32r: the scores matmuls then self-load weights
            # (no 50ns-SEQ Ldweights per matmul, unlike 16-bit dtypes).
            qT_sb = persist.tile([P, 2, T], FR)
            kT_sb = persist.tile([P, 2, T], FR)
            v_sb = persist.tile([P, NH, TC, DH + 1], BF)
            yT_sb = persist.tile([P, 2, T], BF)

            x_tiles = {}

            def load_x(j):
                x_sb = xpool.tile([P, DC, 2, 512], F8, tag="x", name=f"x_{j}")
                nc.sync.dma_start(x_sb[:], xd[:, :, :, 512 * j : 512 * (j + 1)])
                x_tiles[j] = x_sb

            # Startup preload: few big DMAs (each dma_start costs ~625ns of
            # exclusive HWDGE on top of the transfer, so per-chunk interleave
            # makes the first projection DMA-latency-paced).
            x0_sb = xpool.tile([P, DC, 2, 512], F8, tag="x", name="x_0")
            x_tiles[0] = x0_sb
            nc.sync.dma_start(x0_sb[:, 0:4], xd[:, 0:4, :, 0:512])
            nc.sync.dma_start(wq_sb[:], wq)
            nc.sync.dma_start(x0_sb[:, 4:8], xd[:, 4:8, :, 0:512])
            nc.sync.dma_start(wk_sb[:], wk)
            nc.sync.dma_start(wv_sb[:], wv)
            nc.sync.dma_start(mask_sb[:], mask)
            nc.sync.dma_start(id_sb[:], ident)
            load_x(1)
            nc.sync.dma_start(wo_sb[:], wo)

            # ones column of V (softmax sums); gpsimd memset is SBUF-only.
            nc.gpsimd.memset(v_sb[:, :, :, DH : DH + 1], 1.0)

            # --- projection units (fp8 residual DoubleRow) ---
            def proj_qk_unit(j, w_sb, dst, c):
                jsl = slice(512 * j, 512 * (j + 1))
                x_sb = x_tiles[j]
                pt = psum.tile([P, 512], F32, tag="ao", bufs=2, name=f"pqk_{j}_{c}")
                n_i = DC + DC // 2
                k = 0
                for o in range(DC):
                    nc.tensor.matmul(
                        pt[:],
                        w_sb[:, o, :, 128 * c : 128 * (c + 1)],
                        x_sb[:, o, 0, None, :].to_broadcast([P, 2, 512]),
                        start=(k == 0), stop=(k == n_i - 1), perf_mode=DR,
                    )
                    k += 1
                for o in range(0, DC, 2):
                    nc.tensor.matmul(
                        pt[:],
                        w_sb[:, o : o + 2, 0, 128 * c : 128 * (c + 1)],
                        x_sb[:, o : o + 2, 1, :],
                        start=(k == 0), stop=(k == n_i - 1), perf_mode=DR,
                    )
                    k += 1
                nc.vector.tensor_copy(dst[:, c, jsl], pt[:])

            def proj_v_unit(j, i):
                x_sb = x_tiles[j]
                pt = psum.tile([P, 512], F32, tag="ao", bufs=2, name=f"pv_{j}_{i}")
                n_i = DC + DC // 2
                k = 0
                for o in range(DC):
                    nc.tensor.matmul(
                        pt[:, :F],
                        x_sb[:, o, :, 128 * i : 128 * (i + 1)],
                        wv_sb[:, o, 0, None, :].to_broadcast([P, 2, F]),
                        start=(k == 0), stop=(k == n_i - 1), perf_mode=DR,
                    )
                    k += 1
                for o in range(0, DC, 2):
                    nc.tensor.matmul(
                        pt[:, :F],
                        x_sb[:, o : o + 2, 0, 128 * i : 128 * (i + 1)],
                        wv_sb[:, o : o + 2, 1, :],
                        start=(k == 0), stop=(k == n_i - 1), perf_mode=DR,
                    )
                    k += 1
                nc.vector.tensor_copy(
                    v_sb[:, :, 4 * j + i, 0:DH],
                    pt[:, :F].rearrange("p (h d) -> p h d", h=NH),
                )

            def outproj_unit(j, tb, eb):
                while pend_t:
                    flush_transpose()
                pt = psum.tile([P, 512], F32, tag="ao", bufs=2, name=f"o_{j}_{tb}_{eb}")
                for g in range(2):
                    nc.tensor.matmul(
                        pt[:],
                        yT_sb[:, g, 128 * (4 * j + tb) : 128 * (4 * j + tb + 1)],
                        wo_sb[:, g, 512 * eb : 512 * (eb + 1)],
                        start=(g == 0), stop=(g == 1),
                    )
                osb = opool.tile([P, 512], BF, tag="osb", name=f"osb_{j}_{tb}_{eb}")
                if j == TJ - 1:
                    nc.scalar.mul(osb[:], pt[:], OSCALE)
                else:
                    nc.vector.tensor_scalar_mul(osb[:], pt[:], OSCALE)
                nc.sync.dma_start(
                    out[
                        128 * (4 * j + tb) : 128 * (4 * j + tb + 1),
                        512 * eb : 512 * (eb + 1),
                    ],
                    osb[:],
                )

            # --- attention emitters ---
            se_map = {}

            def s_pair(j, h, pp):
                c, tt = h >> 1, h & 1
                hp = DH * tt
                jsl = slice(512 * j, 512 * (j + 1))
                sps = psum.tile([P, 2, 512], F32, tag="sp", bufs=2,
                                name=f"sp_{j}_{h}_{pp}")
                sep = sepool.tile([P, 2, 512], BF, tag="sep", bufs=40,
                                  name=f"sep_{j}_{h}_{pp}")
                for q in range(2):
                    ii = 2 * pp + q
                    nc.tensor.matmul(
                        sps[:, q, :],
                        kT_sb[hp : hp + DH, c, 128 * ii : 128 * (ii + 1)],
                        qT_sb[hp : hp + DH, c, jsl],
                        start=True, stop=True,
                    )
                    se_map[(j, h, ii)] = (sep, q)
                nc.scalar.activation(sep[:], sps[:], AF.Exp, scale=scale)

            def s_diag2(j, c, r):
                # Both heads of c-pair share the diagonal width, so their two
                # S chunks go in one "sp" pair slot and get ONE exp + ONE
                # (broadcast-mask) multiply.
                ii = 4 * j + r
                col0 = 128 * r
                sdp = psum.tile([P, 2, 512], F32, tag="sp", bufs=2,
                                name=f"sd_{j}_{c}_{r}")
                sed = sepool.tile([P, 2, 512], BF, tag="sep", bufs=40,
                                  name=f"sed_{j}_{c}_{r}")
                # fp32r matmuls with N<256 cost 4 cycles/row; widen r=3 to
                # N=256 (extra columns land in never-read se space).
                mcol0 = min(col0, 512 - 256)
                for tt in range(2):
                    hp = DH * tt
                    nc.tensor.matmul(
                        sdp[:, tt, mcol0:],
                        kT_sb[hp : hp + DH, c, 128 * ii : 128 * (ii + 1)],
                        qT_sb[hp : hp + DH, c, 512 * j + mcol0 : 512 * (j + 1)],
                        start=True, stop=True,
                    )
                    se_map[(j, 2 * c + tt, ii)] = (sed, tt)
                if dump_debug and (j, c, r) == (0, 0, 0):
                    spc = small.tile([P, 2, 512], F32, tag="dbgsp", name="spc")
                    nc.vector.tensor_copy(spc[:], sdp[:])
                    nc.sync.dma_start(dbg_sp, spc[:])
                nc.scalar.activation(
                    sed[:, :, col0:], sdp[:, :, col0:], AF.Exp, scale=scale
                )
                nc.gpsimd.tensor_mul(
                    sed[:, :, col0 : col0 + 128],
                    sed[:, :, col0 : col0 + 128],
                    mask_sb[:, r, None, col0 : col0 + 128].to_broadcast([P, 2, 128]),
                )
                if dump_debug and (j, c, r) == (0, 0, 0):
                    nc.sync.dma_start(dbg_se, sed[:])
                if dump_debug and (j, c, r) == (0, 0, 1):
                    nc.sync.dma_start(dbg_se1, sed[:])

            pend_t = []

            def flush_transpose():
                jj, h, b, yf = pend_t.pop(0)
                c, tt = h >> 1, h & 1
                hp = DH * tt
                pt = psum.tile([P, P], BF, tag="ao", bufs=2, name=f"t_{jj}_{h}_{b}")
                nc.tensor.transpose(pt[hp : hp + DH, :], yf[:], id_sb[:])
                nc.vector.tensor_copy(
                    yT_sb[hp : hp + DH, c, 512 * jj + 128 * b : 512 * jj + 128 * (b + 1)],
                    pt[hp : hp + DH, :],
                )

            def pv_unit(j, h, b):
                yps = psum.tile([P, DH + 1], F32, tag="y", bufs=2,
                                name=f"y_{j}_{h}_{b}")
                chunks = list(range(4 * j)) + [4 * j + r for r in range(b + 1)]
                for idx, ii in enumerate(chunks):
                    sep, half = se_map[(j, h, ii)]
                    st = (sep[:, half, 128 * b : 128 * (b + 1)]
                          if half is not None
                          else sep[:, 128 * b : 128 * (b + 1)])
                    nc.tensor.matmul(
                        yps[:], st, v_sb[:, h, ii, :],
                        start=(idx == 0), stop=(idx == len(chunks) - 1),
                    )
                rec = small.tile([P, 1], F32, tag="rec", name=f"rec_{j}_{h}_{b}")
                nc.vector.reciprocal(rec[:], yps[:, DH : DH + 1])
                yf = small.tile([P, DH], BF, tag="yf", name=f"yf_{j}_{h}_{b}")
                nc.vector.tensor_scalar_mul(yf[:], yps[:, 0:DH], rec[:])
                pend_t.append((j, h, b, yf))
                if len(pend_t) > 1:
                    flush_transpose()

            # --- schedule ---
            # Filler distribution targets per-slice PE ~= per-slice ACT (exp
            # grows linearly with j, so out-projections are deferred to the
            # later, exp-heavy slices).
            def proj_units(j):
                u = []
                for c in range(2):
                    u.append(lambda c=c: proj_qk_unit(j, wq_sb, qT_sb, c))
                    u.append(lambda c=c: proj_qk_unit(j, wk_sb, kT_sb, c))
                for i in range(4):
                    u.append(lambda i=i: proj_v_unit(j, i))
                return u

            def outproj_units(j):
                return [
                    lambda tb=tb, eb=eb: outproj_unit(j, tb, eb)
                    for tb in range(4)
                    for eb in range(2)
                ]

            p0 = proj_units(0)
            p0[0]()  # q c0
            p0[2]()  # k c0
            p0[1]()  # q c1
            p0[3]()  # k c1
            p3 = proj_units(3)
            slice_fillers = {
                0: [p0[4], p0[5], p0[6], p0[7]] + proj_units(1),
                1: proj_units(2),
                2: [p3[0], p3[2], p3[4], p3[5]] + outproj_units(0),
                3: [p3[1], p3[3], p3[6], p3[7]] + outproj_units(1) + outproj_units(2)[:4],
            }
            tail_reserve = outproj_units(2)[4:]

            pv_queue = []
            for j in range(TJ):
                fillers = slice_fillers[j]
                if j + 2 < TJ:
                    load_x(j + 2)

                tick = [0]

                def pump():
                    # Alternate filler/PV so deadline-ordered fillers (q/k
                    # c1 before head 2 of slice 0; projections feeding the
                    # next slice) drain even while PV units are queued.
                    tick[0] ^= 1
                    if fillers and (tick[0] or not pv_queue):
                        fillers.pop(0)()
                    elif pv_queue:
                        jj, h, b = pv_queue.pop(0)
                        pv_unit(jj, h, b)

                while pv_queue and pv_queue[0][0] < j - 1:
                    jj, h, b = pv_queue.pop(0)
                    pv_unit(jj, h, b)
                for c in range(2):
                    if c == 1:
                        # Old-slice c0-head PV units must emit before this
                        # slice's c1 S-tiles rotate into their se slots (40
                        # slot rotation; reads emitted after the overwriting
                        # write would see the new data). c1-head units are
                        # safe until the next slice starts.
                        keep = []
                        while pv_queue and pv_queue[0][0] < j:
                            jj, h, b = pv_queue.pop(0)
                            if h < 2:
                                pv_unit(jj, h, b)
                            else:
                                keep.append((jj, h, b))
                        pv_queue[0:0] = keep
                    for tt in range(2):
                        for pp in range(2 * j):
                            s_pair(j, 2 * c + tt, pp)
                            pump()
                    for r in range(4):
                        s_diag2(j, c, r)
                        pump()
                    pv_queue.extend((j, 2 * c + tt, b) for b in range(4) for tt in range(2))

                if j < TJ - 1:
                    while fillers:
                        pump()
                else:
                    # Tail: weave the last slice's output projection in as
                    # each tq-block's final transpose lands.
                    fillers.extend(tail_reserve)
                    while pv_queue:
                        jj, h, b = pv_queue.pop(0)
                        pv_unit(jj, h, b)
                        if fillers:
                            fillers.pop(0)()
                        if jj == TJ - 1 and h == NH - 1 and b > 0:
                            while pend_t and pend_t[0][2] < b:
                                flush_transpose()
                            outproj_unit(j, b - 1, 0)
                            outproj_unit(j, b - 1, 1)
                    while pend_t:
                        flush_transpose()
                    while fillers:
                        fillers.pop(0)()
                    outproj_unit(j, 3, 0)
                    outproj_unit(j, 3, 1)

            if dump_debug:
                nc.sync.dma_start(dbg_q, qT_sb[:])
                nc.sync.dma_start(dbg_k, kT_sb[:])
                nc.sync.dma_start(dbg_v, v_sb[:])
                nc.sync.dma_start(dbg_y, yT_sb[:])
    # move_matmul_waits_to_ldweights moves a matmul's sem waits to "the most
    # recent Ldweights". fp32r matmuls have no Ldweights of their own, so in
    # this mixed fp8/fp32r kernel the pass relocates their RAW waits onto an
    # unrelated earlier fp8 Ldweights, dropping the ordering (observed: the
    # first scores matmul reading q/k before the projection drains). Disable.
    nc.move_matmul_waits_to_ldweights = lambda: None
    nc.compile()
    return nc


def make_mask() -> np.ndarray:
    q = np.arange(512)[None, None, :]
    p = np.arange(P)[:, None, None]
    r = np.arange(4)[None, :, None]
    return (q >= 128 * r + p).astype(ml_dtypes.bfloat16)


def _hilo(a: np.ndarray):
    hi = a.astype(ml_dtypes.float8_e4m3)
    lo = (a - hi.astype(np.float32)).astype(ml_dtypes.float8_e4m3)
    return hi, lo


def _chunked_hilo(a: np.ndarray, width: int, pre_scale: float = 1.0):
    """[D, width] f32 -> [P, DC, 2, width] fp8 (hi/lo interleaved)."""
    hi, lo = _hilo(a * pre_scale)
    s = np.stack([hi.reshape(DC, P, width), lo.reshape(DC, P, width)], axis=2)
    return np.ascontiguousarray(s.transpose(1, 0, 2, 3))


def shard_inputs(x, Wqkv, Wout):
    mask = make_mask()
    identity = np.eye(P).astype(ml_dtypes.bfloat16)
    in_maps = []
    for c in range(N_CORES):
        b, g = c // 4, c % 4
        sl = slice(F * g, F * (g + 1))
        xT = np.ascontiguousarray(x[b].T)  # [D, T]
        woT = np.ascontiguousarray(Wout[:, sl].T).astype(ml_dtypes.bfloat16)  # [F, D]
        in_maps.append(
            {
                "xd": _chunked_hilo(xT, T),
                "wq": _chunked_hilo(np.ascontiguousarray(Wqkv[sl, :].T), F, 16.0),
                "wk": _chunked_hilo(np.ascontiguousarray(Wqkv[D:][sl, :].T), F, 16.0),
                "wv": _chunked_hilo(np.ascontiguousarray(Wqkv[2 * D:][sl, :].T), F, 16.0),
                "wo": np.ascontiguousarray(woT.reshape(2, P, D).transpose(1, 0, 2)),
                "mask": mask,
                "ident": identity,
            }
        )
    return in_maps


_NC_CACHE = None


def kernel(x, Wqkv, Wout):
    global _NC_CACHE
    x = np.asarray(x, dtype=np.float32)
    Wqkv = np.asarray(Wqkv, dtype=np.float32)
    Wout = np.asarray(Wout, dtype=np.float32)
    if _NC_CACHE is None:
        _NC_CACHE = build()
    nc = _NC_CACHE
    in_maps = shard_inputs(x, Wqkv, Wout)
    res = run_bass_kernel_spmd(nc, in_maps, core_ids=list(range(N_CORES)))
    outs = [res.results[c]["out"].astype(np.float32) for c in range(N_CORES)]
    return np.stack(
        [outs[0] + outs[1] + outs[2] + outs[3], outs[4] + outs[5] + outs[6] + outs[7]]
    )


# revision 6
# speedup vs baseline: 1.0587x; 1.0207x over previous
"""Multi-head causal self-attention (B=2, T=2048, D=1024, H=16, Dh=64) on 8 TRN2 cores.

Sharding: data-parallel over batch (2 groups of 4 cores), tensor-parallel over
heads within a group (4 heads/core). Host sums the 4 partial outputs per batch.

v2 design (per core), driven by the TimelineSim cost model (matmul cost =
out-free-size x cycles/row; fp8e4+DoubleRow = 0.5, bf16 = 1.0 at any N):
  - QKV projections run in fp8 with an exact-to-~0.2% residual scheme:
    x is shipped as x8 + xr8 (fp8 value + fp8 residual), W as hi + lo fp8.
    Per d-chunk: one DoubleRow matmul computes (Whi+Wlo)@x8 (hi/lo stacked in
    the two k-tiles); per chunk-pair one DoubleRow matmul computes Whi@xr8.
    => 0.75 cycles/row instead of 1.0, with bf16-class accuracy.
  - Scores S^T = K^T Q per 128-chunk in bf16 (charge = S elements / 128).
    exp on ACT in chunk-pairs ([128,1024] per instruction) to amortize the
    ~370ns per-instruction SBUF-access overhead; diagonal chunks get
    column-trimmed singles + a Pool-engine triangular mask multiply.
  - PV is FLIPPED: out y[tq=128, dh+1] with se as stationary and v (+ones
    column) as moving => charge 65/chunk instead of 512 (bf16 has no N>=256
    requirement). Accumulation is per (head, tq-block) over tk chunks; causal
    skips diagonal chunks r > b. Softmax sums come from the ones column; DVE
    reciprocal + per-partition tensor_scalar multiply normalizes and casts to
    bf16; a PE transpose (vs shipped identity) restores yT for the output
    projection; DVE drains the transposed tile to SBUF.
  - Output projection in bf16; output DMA'd as bf16 and upcast on host.
  - Engine budget: PE ~93us, ACT (exp) ~79us, DVE (drains+normalize) ~70us,
    Pool (masks) ~25us. PE instructions are emitted manually interleaved
    (S-blocks / PV units / projection units) so PSUM WAR chains never
    head-of-line block the PE queue.  GPSIMD cannot touch PSUM, so all PSUM
    drains are on DVE/ACT.
"""
import sys

import numpy as np
import ml_dtypes

for _p in ("/opt/trn_rl_repo", "/root/.axon_site/_ro/trn_rl_repo"):
    if _p not in sys.path:
        try:
            import concourse  # noqa: F401
            break
        except ImportError:
            sys.path.append(_p)

import concourse.bass as bass  # noqa: E402
import concourse.tile as tile  # noqa: E402
from concourse import bacc, mybir  # noqa: E402
from concourse.bass_utils import run_bass_kernel_spmd  # noqa: E402

P = 128
T = 2048
D = 1024
NH = 4          # heads per core
DH = 64
F = NH * DH     # per-core head features (256)
DC = D // P     # 8 contraction chunks
TJ = T // 512   # 4 tq slices
TC = T // P     # 16 tk chunks
N_CORES = 8
F8 = mybir.dt.float8e4
BF = mybir.dt.bfloat16
F32 = mybir.dt.float32
FR = mybir.dt.float32r
DR = mybir.MatmulPerfMode.DoubleRow
AF = mybir.ActivationFunctionType


def build(dump_debug=False):
    nc = bacc.Bacc("TRN2", target_bir_lowering=False, debug=False, num_devices=N_CORES)
    xd = nc.dram_tensor("xd", [P, DC, 2, T], F8, kind="ExternalInput").ap()
    wq = nc.dram_tensor("wq", [P, DC, 2, F], F8, kind="ExternalInput").ap()
    wk = nc.dram_tensor("wk", [P, DC, 2, F], F8, kind="ExternalInput").ap()
    wv = nc.dram_tensor("wv", [P, DC, 2, F], F8, kind="ExternalInput").ap()
    wo = nc.dram_tensor("wo", [P, 2, D], BF, kind="ExternalInput").ap()
    mask = nc.dram_tensor("mask", [P, 4, 512], BF, kind="ExternalInput").ap()
    ident = nc.dram_tensor("ident", [P, P], BF, kind="ExternalInput").ap()
    out = nc.dram_tensor("out", [T, D], BF, kind="ExternalOutput").ap()
    if dump_debug:
        dbg_q = nc.dram_tensor("dbg_q", [P, 2, T], FR, kind="ExternalOutput").ap()
        dbg_k = nc.dram_tensor("dbg_k", [P, 2, T], FR, kind="ExternalOutput").ap()
        dbg_v = nc.dram_tensor("dbg_v", [P, NH, TC, DH + 1], BF, kind="ExternalOutput").ap()
        dbg_y = nc.dram_tensor("dbg_y", [P, 2, T], BF, kind="ExternalOutput").ap()
        dbg_se = nc.dram_tensor("dbg_se", [P, 2, 512], BF, kind="ExternalOutput").ap()
        dbg_se1 = nc.dram_tensor("dbg_se1", [P, 2, 512], BF, kind="ExternalOutput").ap()
        dbg_sp = nc.dram_tensor("dbg_sp", [P, 2, 512], F32, kind="ExternalOutput").ap()

    # Wq/Wk/Wv are host-scaled by 16 so their fp8 residuals don't underflow
    # e4m3's exponent range; q.k then carries 256x, absorbed into the exp
    # scale. v carries 16x, absorbed in the output-projection drain.
    scale = 1.0 / (np.sqrt(DH) * 256.0)
    OSCALE = 1.0 / 16.0

    with tile.TileContext(nc) as tc:
        with (
            tc.tile_pool(name="weights", bufs=1) as wpool,
            tc.tile_pool(name="persist", bufs=1) as persist,
            tc.tile_pool(name="x", bufs=2) as xpool,
            tc.tile_pool(name="sexp", bufs=1) as sepool,
            tc.tile_pool(name="small", bufs=4) as small,
            tc.tile_pool(name="outsb", bufs=4) as opool,
            tc.tile_pool(name="ps", bufs=1, space="PSUM") as psum,
        ):
            wq_sb = wpool.tile([P, DC, 2, F], F8)
            wk_sb = wpool.tile([P, DC, 2, F], F8)
            wv_sb = wpool.tile([P, DC, 2, F], F8)
            wo_sb = wpool.tile([P, 2, D], BF)
            mask_sb = wpool.tile([P, 4, 512], BF)
            id_sb = wpool.tile([P, P], BF)

            # q/k live in fp32r: the scores matmuls then self-load weights
            # (no 50ns-SEQ Ldweights per matmul, unlike 16-bit dtypes).
            qT_sb = persist.tile([P, 2, T], FR)
            kT_sb = persist.tile([P, 2, T], FR)
            v_sb = persist.tile([P, NH, TC, DH + 1], BF)
            yT_sb = persist.tile([P, 2, T], BF)

            x_tiles = {}

            def load_x(j):
                x_sb = xpool.tile([P, DC, 2, 512], F8, tag="x", name=f"x_{j}")
                nc.sync.dma_start(x_sb[:], xd[:, :, :, 512 * j : 512 * (j + 1)])
                x_tiles[j] = x_sb

            # Startup preload: few big DMAs (each dma_start costs ~625ns of
            # exclusive HWDGE on top of the transfer, so per-chunk interleave
            # makes the first projection DMA-latency-paced).
            x0_sb = xpool.tile([P, DC, 2, 512], F8, tag="x", name="x_0")
            x_tiles[0] = x0_sb
            # x8 plane first: projection I1 matmuls need only x8; the xr8
            # residual plane (used by the trailing I2 matmuls) follows wk.
            nc.sync.dma_start(wq_sb[:], wq)
            nc.sync.dma_start(x0_sb[:, :, 0:1], xd[:, :, 0:1, 0:512])
            nc.sync.dma_start(wk_sb[:], wk)
            nc.sync.dma_start(x0_sb[:, :, 1:2], xd[:, :, 1:2, 0:512])
            nc.sync.dma_start(wv_sb[:], wv)
            nc.sync.dma_start(mask_sb[:], mask)
            nc.sync.dma_start(id_sb[:], ident)
            load_x(1)
            nc.sync.dma_start(wo_sb[:], wo)

            # ones column of V (softmax sums); gpsimd memset is SBUF-only.
            nc.gpsimd.memset(v_sb[:, :, :, DH : DH + 1], 1.0)

            # --- projection units (fp8 residual DoubleRow) ---
            def proj_qk_unit(j, w_sb, dst, c):
                jsl = slice(512 * j, 512 * (j + 1))
                x_sb = x_tiles[j]
                pt = psum.tile([P, 512], F32, tag="ao", bufs=2, name=f"pqk_{j}_{c}")
                n_i = DC + DC // 2
                k = 0
                for o in range(DC):
                    nc.tensor.matmul(
                        pt[:],
                        w_sb[:, o, :, 128 * c : 128 * (c + 1)],
                        x_sb[:, o, 0, None, :].to_broadcast([P, 2, 512]),
                        start=(k == 0), stop=(k == n_i - 1), perf_mode=DR,
                    )
                    k += 1
                for o in range(0, DC, 2):
                    nc.tensor.matmul(
                        pt[:],
                        w_sb[:, o : o + 2, 0, 128 * c : 128 * (c + 1)],
                        x_sb[:, o : o + 2, 1, :],
                        start=(k == 0), stop=(k == n_i - 1), perf_mode=DR,
                    )
                    k += 1
                nc.vector.tensor_copy(dst[:, c, jsl], pt[:])

            def proj_v_unit(j, i):
                x_sb = x_tiles[j]
                pt = psum.tile([P, 512], F32, tag="ao", bufs=2, name=f"pv_{j}_{i}")
                n_i = DC + DC // 2
                k = 0
                for o in range(DC):
                    nc.tensor.matmul(
                        pt[:, :F],
                        x_sb[:, o, :, 128 * i : 128 * (i + 1)],
                        wv_sb[:, o, 0, None, :].to_broadcast([P, 2, F]),
                        start=(k == 0), stop=(k == n_i - 1), perf_mode=DR,
                    )
                    k += 1
                for o in range(0, DC, 2):
                    nc.tensor.matmul(
                        pt[:, :F],
                        x_sb[:, o : o + 2, 0, 128 * i : 128 * (i + 1)],
                        wv_sb[:, o : o + 2, 1, :],
                        start=(k == 0), stop=(k == n_i - 1), perf_mode=DR,
                    )
                    k += 1
                nc.vector.tensor_copy(
                    v_sb[:, :, 4 * j + i, 0:DH],
                    pt[:, :F].rearrange("p (h d) -> p h d", h=NH),
                )

            def outproj_unit(j, tb, eb):
                while pend_t:
                    flush_transpose()
                pt = psum.tile([P, 512], F32, tag="ao", bufs=2, name=f"o_{j}_{tb}_{eb}")
                for g in range(2):
                    nc.tensor.matmul(
                        pt[:],
                        yT_sb[:, g, 128 * (4 * j + tb) : 128 * (4 * j + tb + 1)],
                        wo_sb[:, g, 512 * eb : 512 * (eb + 1)],
                        start=(g == 0), stop=(g == 1),
                    )
                osb = opool.tile([P, 512], BF, tag="osb", name=f"osb_{j}_{tb}_{eb}")
                rows = slice(128 * (4 * j + tb), 128 * (4 * j + tb + 1))
                if j == TJ - 1 and tb == 3:
                    for hf in range(2):
                        cs = slice(256 * hf, 256 * (hf + 1))
                        nc.scalar.mul(osb[:, cs], pt[:, cs], OSCALE)
                        nc.sync.dma_start(
                            out[rows, 512 * eb + 256 * hf : 512 * eb + 256 * (hf + 1)],
                            osb[:, cs],
                        )
                elif j == TJ - 1:
                    nc.scalar.mul(osb[:], pt[:], OSCALE)
                    nc.sync.dma_start(out[rows, 512 * eb : 512 * (eb + 1)], osb[:])
                else:
                    nc.vector.tensor_scalar_mul(osb[:], pt[:], OSCALE)
                    nc.sync.dma_start(out[rows, 512 * eb : 512 * (eb + 1)], osb[:])

            # --- attention emitters ---
            se_map = {}

            def s_pair(j, h, pp):
                c, tt = h >> 1, h & 1
                hp = DH * tt
                jsl = slice(512 * j, 512 * (j + 1))
                sps = psum.tile([P, 2, 512], F32, tag="sp", bufs=2,
                                name=f"sp_{j}_{h}_{pp}")
                sep = sepool.tile([P, 2, 512], BF, tag="sep", bufs=40,
                                  name=f"sep_{j}_{h}_{pp}")
                for q in range(2):
                    ii = 2 * pp + q
                    nc.tensor.matmul(
                        sps[:, q, :],
                        kT_sb[hp : hp + DH, c, 128 * ii : 128 * (ii + 1)],
                        qT_sb[hp : hp + DH, c, jsl],
                        start=True, stop=True,
                    )
                    se_map[(j, h, ii)] = (sep, q)
                nc.scalar.activation(sep[:], sps[:], AF.Exp, scale=scale)

            def s_diag2(j, c, r):
                # Both heads of c-pair share the diagonal width, so their two
                # S chunks go in one "sp" pair slot and get ONE exp + ONE
                # (broadcast-mask) multiply.
                ii = 4 * j + r
                col0 = 128 * r
                sdp = psum.tile([P, 2, 512], F32, tag="sp", bufs=2,
                                name=f"sd_{j}_{c}_{r}")
                sed = sepool.tile([P, 2, 512], BF, tag="sep", bufs=40,
                                  name=f"sed_{j}_{c}_{r}")
                # fp32r matmuls with N<256 cost 4 cycles/row; widen r=3 to
                # N=256 (extra columns land in never-read se space).
                mcol0 = min(col0, 512 - 256)
                for tt in range(2):
                    hp = DH * tt
                    nc.tensor.matmul(
                        sdp[:, tt, mcol0:],
                        kT_sb[hp : hp + DH, c, 128 * ii : 128 * (ii + 1)],
                        qT_sb[hp : hp + DH, c, 512 * j + mcol0 : 512 * (j + 1)],
                        start=True, stop=True,
                    )
                    se_map[(j, 2 * c + tt, ii)] = (sed, tt)
                if dump_debug and (j, c, r) == (0, 0, 0):
                    spc = small.tile([P, 2, 512], F32, tag="dbgsp", name="spc")
                    nc.vector.tensor_copy(spc[:], sdp[:])
                    nc.sync.dma_start(dbg_sp, spc[:])
                nc.scalar.activation(
                    sed[:, :, col0:], sdp[:, :, col0:], AF.Exp, scale=scale
                )
                nc.gpsimd.tensor_mul(
                    sed[:, :, col0 : col0 + 128],
                    sed[:, :, col0 : col0 + 128],
                    mask_sb[:, r, None, col0 : col0 + 128].to_broadcast([P, 2, 128]),
                )
                if dump_debug and (j, c, r) == (0, 0, 0):
                    nc.sync.dma_start(dbg_se, sed[:])
                if dump_debug and (j, c, r) == (0, 0, 1):
                    nc.sync.dma_start(dbg_se1, sed[:])

            pend_t = []

            def flush_transpose():
                jj, h, b, yf = pend_t.pop(0)
                c, tt = h >> 1, h & 1
                hp = DH * tt
                pt = psum.tile([P, P], BF, tag="ao", bufs=2, name=f"t_{jj}_{h}_{b}")
                nc.tensor.transpose(pt[hp : hp + DH, :], yf[:], id_sb[:])
                nc.vector.tensor_copy(
                    yT_sb[hp : hp + DH, c, 512 * jj + 128 * b : 512 * jj + 128 * (b + 1)],
                    pt[hp : hp + DH, :],
                )

            def pv_unit(j, h, b):
                yps = psum.tile([P, DH + 1], F32, tag="y", bufs=2,
                                name=f"y_{j}_{h}_{b}")
                chunks = list(range(4 * j)) + [4 * j + r for r in range(b + 1)]
                for idx, ii in enumerate(chunks):
                    sep, half = se_map[(j, h, ii)]
                    st = (sep[:, half, 128 * b : 128 * (b + 1)]
                          if half is not None
                          else sep[:, 128 * b : 128 * (b + 1)])
                    nc.tensor.matmul(
                        yps[:], st, v_sb[:, h, ii, :],
                        start=(idx == 0), stop=(idx == len(chunks) - 1),
                    )
                rec = small.tile([P, 1], F32, tag="rec", name=f"rec_{j}_{h}_{b}")
                nc.vector.reciprocal(rec[:], yps[:, DH : DH + 1])
                yf = small.tile([P, DH], BF, tag="yf", name=f"yf_{j}_{h}_{b}")
                nc.vector.tensor_scalar_mul(yf[:], yps[:, 0:DH], rec[:])
                pend_t.append((j, h, b, yf))
                if len(pend_t) > 1:
                    flush_transpose()

            # --- schedule ---
            # Filler distribution targets per-slice PE ~= per-slice ACT (exp
            # grows linearly with j, so out-projections are deferred to the
            # later, exp-heavy slices).
            def proj_units(j):
                u = []
                for c in range(2):
                    u.append(lambda c=c: proj_qk_unit(j, wq_sb, qT_sb, c))
                    u.append(lambda c=c: proj_qk_unit(j, wk_sb, kT_sb, c))
                for i in range(4):
                    u.append(lambda i=i: proj_v_unit(j, i))
                return u

            def outproj_units(j):
                return [
                    lambda tb=tb, eb=eb: outproj_unit(j, tb, eb)
                    for tb in range(4)
                    for eb in range(2)
                ]

            p0 = proj_units(0)
            p0[0]()  # q c0
            p0[2]()  # k c0
            p0[1]()  # q c1
            p0[3]()  # k c1
            p3 = proj_units(3)
            slice_fillers = {
                0: [p0[4], p0[5], p0[6], p0[7]] + proj_units(1),
                1: proj_units(2),
                2: [p3[0], p3[2], p3[4], p3[5]] + outproj_units(0),
                3: [p3[1], p3[3], p3[6], p3[7]] + outproj_units(1) + outproj_units(2)[:4],
            }
            tail_reserve = outproj_units(2)[4:]

            pv_queue = []
            for j in range(TJ):
                fillers = slice_fillers[j]
                if j + 2 < TJ:
                    load_x(j + 2)

                tick = [0]

                def pump():
                    # Alternate filler/PV so deadline-ordered fillers (q/k
                    # c1 before head 2 of slice 0; projections feeding the
                    # next slice) drain even while PV units are queued.
                    tick[0] ^= 1
                    if fillers and (tick[0] or not pv_queue):
                        fillers.pop(0)()
                    elif pv_queue:
                        jj, h, b = pv_queue.pop(0)
                        pv_unit(jj, h, b)

                while pv_queue and pv_queue[0][0] < j - 1:
                    jj, h, b = pv_queue.pop(0)
                    pv_unit(jj, h, b)
                for c in range(2):
                    if c == 1:
                        # Old-slice c0-head PV units must emit before this
                        # slice's c1 S-tiles rotate into their se slots (40
                        # slot rotation; reads emitted after the overwriting
                        # write would see the new data). c1-head units are
                        # safe until the next slice starts.
                        keep = []
                        while pv_queue and pv_queue[0][0] < j:
                            jj, h, b = pv_queue.pop(0)
                            if h < 2:
                                pv_unit(jj, h, b)
                            else:
                                keep.append((jj, h, b))
                        pv_queue[0:0] = keep
                    for tt in range(2):
                        for pp in range(2 * j):
                            s_pair(j, 2 * c + tt, pp)
                            pump()
                    for r in range(4):
                        s_diag2(j, c, r)
                        pump()
                    pv_queue.extend((j, 2 * c + tt, b) for b in range(4) for tt in range(2))

                if j < TJ - 1:
                    while fillers:
                        pump()
                else:
                    # Tail: weave the last slice's output projection in as
                    # each tq-block's final transpose lands.
                    fillers.extend(tail_reserve)
                    while pv_queue:
                        jj, h, b = pv_queue.pop(0)
                        pv_unit(jj, h, b)
                        if fillers:
                            fillers.pop(0)()
                        if jj == TJ - 1 and h == NH - 1 and b > 0:
                            while pend_t and pend_t[0][2] < b:
                                flush_transpose()
                            outproj_unit(j, b - 1, 0)
                            outproj_unit(j, b - 1, 1)
                    while pend_t:
                        flush_transpose()
                    while fillers:
                        fillers.pop(0)()
                    outproj_unit(j, 3, 0)
                    outproj_unit(j, 3, 1)

            if dump_debug:
                nc.sync.dma_start(dbg_q, qT_sb[:])
                nc.sync.dma_start(dbg_k, kT_sb[:])
                nc.sync.dma_start(dbg_v, v_sb[:])
                nc.sync.dma_start(dbg_y, yT_sb[:])
    # move_matmul_waits_to_ldweights moves a matmul's sem waits to "the most
    # recent Ldweights". fp32r matmuls have no Ldweights of their own, so in
    # this mixed fp8/fp32r kernel the pass relocates their RAW waits onto an
    # unrelated earlier fp8 Ldweights, dropping the ordering (observed: the
    # first scores matmul reading q/k before the projection drains). Disable.
    nc.move_matmul_waits_to_ldweights = lambda: None
    nc.compile()
    return nc


def make_mask() -> np.ndarray:
    q = np.arange(512)[None, None, :]
    p = np.arange(P)[:, None, None]
    r = np.arange(4)[None, :, None]
    return (q >= 128 * r + p).astype(ml_dtypes.bfloat16)


def _hilo(a: np.ndarray):
    hi = a.astype(ml_dtypes.float8_e4m3)
    lo = (a - hi.astype(np.float32)).astype(ml_dtypes.float8_e4m3)
    return hi, lo


def _chunked_hilo(a: np.ndarray, width: int, pre_scale: float = 1.0):
    """[D, width] f32 -> [P, DC, 2, width] fp8 (hi/lo interleaved)."""
    hi, lo = _hilo(a * pre_scale)
    s = np.stack([hi.reshape(DC, P, width), lo.reshape(DC, P, width)], axis=2)
    return np.ascontiguousarray(s.transpose(1, 0, 2, 3))


def shard_inputs(x, Wqkv, Wout):
    mask = make_mask()
    identity = np.eye(P).astype(ml_dtypes.bfloat16)
    in_maps = []
    for c in range(N_CORES):
        b, g = c // 4, c % 4
        sl = slice(F * g, F * (g + 1))
        xT = np.ascontiguousarray(x[b].T)  # [D, T]
        woT = np.ascontiguousarray(Wout[:, sl].T).astype(ml_dtypes.bfloat16)  # [F, D]
        in_maps.append(
            {
                "xd": _chunked_hilo(xT, T),
                "wq": _chunked_hilo(np.ascontiguousarray(Wqkv[sl, :].T), F, 16.0),
                "wk": _chunked_hilo(np.ascontiguousarray(Wqkv[D:][sl, :].T), F, 16.0),
                "wv": _chunked_hilo(np.ascontiguousarray(Wqkv[2 * D:][sl, :].T), F, 16.0),
                "wo": np.ascontiguousarray(woT.reshape(2, P, D).transpose(1, 0, 2)),
                "mask": mask,
                "ident": identity,
            }
        )
    return in_maps


_NC_CACHE = None


def kernel(x, Wqkv, Wout):
    global _NC_CACHE
    x = np.asarray(x, dtype=np.float32)
    Wqkv = np.asarray(Wqkv, dtype=np.float32)
    Wout = np.asarray(Wout, dtype=np.float32)
    if _NC_CACHE is None:
        _NC_CACHE = build()
    nc = _NC_CACHE
    in_maps = shard_inputs(x, Wqkv, Wout)
    res = run_bass_kernel_spmd(nc, in_maps, core_ids=list(range(N_CORES)))
    outs = [res.results[c]["out"].astype(np.float32) for c in range(N_CORES)]
    return np.stack(
        [outs[0] + outs[1] + outs[2] + outs[3], outs[4] + outs[5] + outs[6] + outs[7]]
    )


# revision 7
# speedup vs baseline: 1.0726x; 1.0131x over previous
"""Multi-head causal self-attention (B=2, T=2048, D=1024, H=16, Dh=64) on 8 TRN2 cores.

Sharding: data-parallel over batch (2 groups of 4 cores), tensor-parallel over
heads within a group (4 heads/core). Host sums the 4 partial outputs per batch.

v2 design (per core), driven by the TimelineSim cost model (matmul cost =
out-free-size x cycles/row; fp8e4+DoubleRow = 0.5, bf16 = 1.0 at any N):
  - QKV projections run in fp8 with an exact-to-~0.2% residual scheme:
    x is shipped as x8 + xr8 (fp8 value + fp8 residual), W as hi + lo fp8.
    Per d-chunk: one DoubleRow matmul computes (Whi+Wlo)@x8 (hi/lo stacked in
    the two k-tiles); per chunk-pair one DoubleRow matmul computes Whi@xr8.
    => 0.75 cycles/row instead of 1.0, with bf16-class accuracy.
  - Scores S^T = K^T Q per 128-chunk in bf16 (charge = S elements / 128).
    exp on ACT in chunk-pairs ([128,1024] per instruction) to amortize the
    ~370ns per-instruction SBUF-access overhead; diagonal chunks get
    column-trimmed singles + a Pool-engine triangular mask multiply.
  - PV is FLIPPED: out y[tq=128, dh+1] with se as stationary and v (+ones
    column) as moving => charge 65/chunk instead of 512 (bf16 has no N>=256
    requirement). Accumulation is per (head, tq-block) over tk chunks; causal
    skips diagonal chunks r > b. Softmax sums come from the ones column; DVE
    reciprocal + per-partition tensor_scalar multiply normalizes and casts to
    bf16; a PE transpose (vs shipped identity) restores yT for the output
    projection; DVE drains the transposed tile to SBUF.
  - Output projection in bf16; output DMA'd as bf16 and upcast on host.
  - Engine budget: PE ~93us, ACT (exp) ~79us, DVE (drains+normalize) ~70us,
    Pool (masks) ~25us. PE instructions are emitted manually interleaved
    (S-blocks / PV units / projection units) so PSUM WAR chains never
    head-of-line block the PE queue.  GPSIMD cannot touch PSUM, so all PSUM
    drains are on DVE/ACT.
"""
import sys

import numpy as np
import ml_dtypes

for _p in ("/opt/trn_rl_repo", "/root/.axon_site/_ro/trn_rl_repo"):
    if _p not in sys.path:
        try:
            import concourse  # noqa: F401
            break
        except ImportError:
            sys.path.append(_p)

import concourse.bass as bass  # noqa: E402
import concourse.tile as tile  # noqa: E402
from concourse import bacc, mybir  # noqa: E402
from concourse.bass_utils import run_bass_kernel_spmd  # noqa: E402

P = 128
T = 2048
D = 1024
NH = 4          # heads per core
DH = 64
F = NH * DH     # per-core head features (256)
DC = D // P     # 8 contraction chunks
TJ = T // 512   # 4 tq slices
TC = T // P     # 16 tk chunks
N_CORES = 8
F8 = mybir.dt.float8e4
BF = mybir.dt.bfloat16
F32 = mybir.dt.float32
FR = mybir.dt.float32r
DR = mybir.MatmulPerfMode.DoubleRow
AF = mybir.ActivationFunctionType


def build(dump_debug=False):
    nc = bacc.Bacc("TRN2", target_bir_lowering=False, debug=False, num_devices=N_CORES)
    xd = nc.dram_tensor("xd", [P, DC, 2, T], F8, kind="ExternalInput").ap()
    wq = nc.dram_tensor("wq", [P, DC, 2, F], F8, kind="ExternalInput").ap()
    wk = nc.dram_tensor("wk", [P, DC, 2, F], F8, kind="ExternalInput").ap()
    wv = nc.dram_tensor("wv", [P, DC, 2, F], F8, kind="ExternalInput").ap()
    wo = nc.dram_tensor("wo", [P, 2, D], BF, kind="ExternalInput").ap()
    mask = nc.dram_tensor("mask", [P, 4, 512], BF, kind="ExternalInput").ap()
    ident = nc.dram_tensor("ident", [P, P], BF, kind="ExternalInput").ap()
    out = nc.dram_tensor("out", [T, D], BF, kind="ExternalOutput").ap()
    if dump_debug:
        dbg_q = nc.dram_tensor("dbg_q", [P, 2, T], FR, kind="ExternalOutput").ap()
        dbg_k = nc.dram_tensor("dbg_k", [P, 2, T], FR, kind="ExternalOutput").ap()
        dbg_v = nc.dram_tensor("dbg_v", [P, NH, TC, DH + 1], BF, kind="ExternalOutput").ap()
        dbg_y = nc.dram_tensor("dbg_y", [P, 2, T], BF, kind="ExternalOutput").ap()
        dbg_se = nc.dram_tensor("dbg_se", [P, 2, 512], BF, kind="ExternalOutput").ap()
        dbg_se1 = nc.dram_tensor("dbg_se1", [P, 2, 512], BF, kind="ExternalOutput").ap()
        dbg_sp = nc.dram_tensor("dbg_sp", [P, 2, 512], F32, kind="ExternalOutput").ap()

    # Wq/Wk/Wv are host-scaled by 16 so their fp8 residuals don't underflow
    # e4m3's exponent range; q.k then carries 256x, absorbed into the exp
    # scale. v carries 16x, absorbed in the output-projection drain.
    scale = 1.0 / (np.sqrt(DH) * 256.0)
    OSCALE = 1.0 / 16.0

    with tile.TileContext(nc) as tc:
        with (
            tc.tile_pool(name="weights", bufs=1) as wpool,
            tc.tile_pool(name="persist", bufs=1) as persist,
            tc.tile_pool(name="x", bufs=2) as xpool,
            tc.tile_pool(name="sexp", bufs=1) as sepool,
            tc.tile_pool(name="small", bufs=4) as small,
            tc.tile_pool(name="outsb", bufs=4) as opool,
            tc.tile_pool(name="ps", bufs=1, space="PSUM") as psum,
        ):
            wq_sb = wpool.tile([P, DC, 2, F], F8)
            wk_sb = wpool.tile([P, DC, 2, F], F8)
            wv_sb = wpool.tile([P, DC, 2, F], F8)
            wo_sb = wpool.tile([P, 2, D], BF)
            mask_sb = wpool.tile([P, 4, 512], BF)
            id_sb = wpool.tile([P, P], BF)

            # q/k live in fp32r: the scores matmuls then self-load weights
            # (no 50ns-SEQ Ldweights per matmul, unlike 16-bit dtypes).
            qT_sb = persist.tile([P, 2, T], FR)
            kT_sb = persist.tile([P, 2, T], FR)
            v_sb = persist.tile([P, NH, TC, DH + 1], BF)
            yT_sb = persist.tile([P, 2, T], BF)

            x_tiles = {}

            def load_x(j):
                x_sb = xpool.tile([P, DC, 2, 512], F8, tag="x", name=f"x_{j}")
                nc.sync.dma_start(x_sb[:], xd[:, :, :, 512 * j : 512 * (j + 1)])
                x_tiles[j] = x_sb

            # Startup preload: few big DMAs (each dma_start costs ~625ns of
            # exclusive HWDGE on top of the transfer, so per-chunk interleave
            # makes the first projection DMA-latency-paced).
            x0_sb = xpool.tile([P, DC, 2, 512], F8, tag="x", name="x_0")
            x_tiles[0] = x0_sb
            # x8 plane first: projection I1 matmuls need only x8; the xr8
            # residual plane (used by the trailing I2 matmuls) follows wk.
            nc.sync.dma_start(wq_sb[:], wq)
            nc.sync.dma_start(x0_sb[:, :, 0:1], xd[:, :, 0:1, 0:512])
            nc.sync.dma_start(wk_sb[:], wk)
            nc.sync.dma_start(x0_sb[:, :, 1:2], xd[:, :, 1:2, 0:512])
            nc.sync.dma_start(wv_sb[:], wv)
            nc.sync.dma_start(mask_sb[:], mask)
            nc.sync.dma_start(id_sb[:], ident)
            load_x(1)
            nc.sync.dma_start(wo_sb[:], wo)

            # ones column of V (softmax sums); gpsimd memset is SBUF-only.
            nc.gpsimd.memset(v_sb[:, :, :, DH : DH + 1], 1.0)

            # --- projection units (fp8 residual DoubleRow) ---
            def proj_qk_unit(j, w_sb, dst, c):
                jsl = slice(512 * j, 512 * (j + 1))
                x_sb = x_tiles[j]
                pt = psum.tile([P, 512], F32, tag="ao", bufs=2, name=f"pqk_{j}_{c}")
                n_i = DC + DC // 2
                k = 0
                for o in range(DC):
                    nc.tensor.matmul(
                        pt[:],
                        w_sb[:, o, :, 128 * c : 128 * (c + 1)],
                        x_sb[:, o, 0, None, :].to_broadcast([P, 2, 512]),
                        start=(k == 0), stop=(k == n_i - 1), perf_mode=DR,
                    )
                    k += 1
                for o in range(0, DC, 2):
                    nc.tensor.matmul(
                        pt[:],
                        w_sb[:, o : o + 2, 0, 128 * c : 128 * (c + 1)],
                        x_sb[:, o : o + 2, 1, :],
                        start=(k == 0), stop=(k == n_i - 1), perf_mode=DR,
                    )
                    k += 1
                nc.vector.tensor_copy(dst[:, c, jsl], pt[:])

            def proj_v_unit(j, i):
                x_sb = x_tiles[j]
                pt = psum.tile([P, 512], F32, tag="ao", bufs=2, name=f"pv_{j}_{i}")
                n_i = DC + DC // 2
                k = 0
                for o in range(DC):
                    nc.tensor.matmul(
                        pt[:, :F],
                        x_sb[:, o, :, 128 * i : 128 * (i + 1)],
                        wv_sb[:, o, 0, None, :].to_broadcast([P, 2, F]),
                        start=(k == 0), stop=(k == n_i - 1), perf_mode=DR,
                    )
                    k += 1
                for o in range(0, DC, 2):
                    nc.tensor.matmul(
                        pt[:, :F],
                        x_sb[:, o : o + 2, 0, 128 * i : 128 * (i + 1)],
                        wv_sb[:, o : o + 2, 1, :],
                        start=(k == 0), stop=(k == n_i - 1), perf_mode=DR,
                    )
                    k += 1
                nc.vector.tensor_copy(
                    v_sb[:, :, 4 * j + i, 0:DH],
                    pt[:, :F].rearrange("p (h d) -> p h d", h=NH),
                )

            def outproj_unit(j, tb, eb):
                while pend_t:
                    flush_transpose()
                pt = psum.tile([P, 512], F32, tag="ao", bufs=2, name=f"o_{j}_{tb}_{eb}")
                for g in range(2):
                    nc.tensor.matmul(
                        pt[:],
                        yT_sb[:, g, 128 * (4 * j + tb) : 128 * (4 * j + tb + 1)],
                        wo_sb[:, g, 512 * eb : 512 * (eb + 1)],
                        start=(g == 0), stop=(g == 1),
                    )
                osb = opool.tile([P, 512], BF, tag="osb", name=f"osb_{j}_{tb}_{eb}")
                rows = slice(128 * (4 * j + tb), 128 * (4 * j + tb + 1))
                if j == TJ - 1 and tb == 3:
                    for hf in range(2):
                        cs = slice(256 * hf, 256 * (hf + 1))
                        nc.scalar.mul(osb[:, cs], pt[:, cs], OSCALE)
                        nc.sync.dma_start(
                            out[rows, 512 * eb + 256 * hf : 512 * eb + 256 * (hf + 1)],
                            osb[:, cs],
                        )
                elif j == TJ - 1:
                    nc.scalar.mul(osb[:], pt[:], OSCALE)
                    nc.sync.dma_start(out[rows, 512 * eb : 512 * (eb + 1)], osb[:])
                else:
                    nc.vector.tensor_scalar_mul(osb[:], pt[:], OSCALE)
                    nc.sync.dma_start(out[rows, 512 * eb : 512 * (eb + 1)], osb[:])

            # --- attention emitters ---
            se_map = {}

            def s_pair(j, h, pp):
                c, tt = h >> 1, h & 1
                hp = DH * tt
                jsl = slice(512 * j, 512 * (j + 1))
                sps = psum.tile([P, 2, 512], F32, tag="sp", bufs=2,
                                name=f"sp_{j}_{h}_{pp}")
                sep = sepool.tile([P, 2, 512], BF, tag="sep", bufs=40,
                                  name=f"sep_{j}_{h}_{pp}")
                for q in range(2):
                    ii = 2 * pp + q
                    nc.tensor.matmul(
                        sps[:, q, :],
                        kT_sb[hp : hp + DH, c, 128 * ii : 128 * (ii + 1)],
                        qT_sb[hp : hp + DH, c, jsl],
                        start=True, stop=True,
                    )
                    se_map[(j, h, ii)] = (sep, q)
                nc.scalar.activation(sep[:], sps[:], AF.Exp, scale=scale)

            def s_diag2(j, c, r):
                # Both heads of c-pair share the diagonal width, so their two
                # S chunks go in one "sp" pair slot and get ONE exp + ONE
                # (broadcast-mask) multiply.
                ii = 4 * j + r
                col0 = 128 * r
                sdp = psum.tile([P, 2, 512], F32, tag="sp", bufs=2,
                                name=f"sd_{j}_{c}_{r}")
                sed = sepool.tile([P, 2, 512], BF, tag="sep", bufs=40,
                                  name=f"sed_{j}_{c}_{r}")
                # fp32r matmuls with N<256 cost 4 cycles/row; widen r=3 to
                # N=256 (extra columns land in never-read se space).
                mcol0 = min(col0, 512 - 256)
                for tt in range(2):
                    hp = DH * tt
                    nc.tensor.matmul(
                        sdp[:, tt, mcol0:],
                        kT_sb[hp : hp + DH, c, 128 * ii : 128 * (ii + 1)],
                        qT_sb[hp : hp + DH, c, 512 * j + mcol0 : 512 * (j + 1)],
                        start=True, stop=True,
                    )
                    se_map[(j, 2 * c + tt, ii)] = (sed, tt)
                if dump_debug and (j, c, r) == (0, 0, 0):
                    spc = small.tile([P, 2, 512], F32, tag="dbgsp", name="spc")
                    nc.vector.tensor_copy(spc[:], sdp[:])
                    nc.sync.dma_start(dbg_sp, spc[:])
                nc.scalar.activation(
                    sed[:, :, col0:], sdp[:, :, col0:], AF.Exp, scale=scale
                )
                nc.gpsimd.tensor_mul(
                    sed[:, :, col0 : col0 + 128],
                    sed[:, :, col0 : col0 + 128],
                    mask_sb[:, r, None, col0 : col0 + 128].to_broadcast([P, 2, 128]),
                )
                if dump_debug and (j, c, r) == (0, 0, 0):
                    nc.sync.dma_start(dbg_se, sed[:])
                if dump_debug and (j, c, r) == (0, 0, 1):
                    nc.sync.dma_start(dbg_se1, sed[:])

            pend_t = []
            pend_yf = {}

            def flush_transpose():
                # One transpose per head-PAIR: yf2 holds both heads' columns,
                # the transpose emits the full 128-partition yT block.
                jj, c, b, yf2 = pend_t.pop(0)
                pt = psum.tile([P, P], BF, tag="ao", bufs=2, name=f"t_{jj}_{c}_{b}")
                nc.tensor.transpose(pt[:], yf2[:], id_sb[:])
                nc.vector.tensor_copy(
                    yT_sb[:, c, 512 * jj + 128 * b : 512 * jj + 128 * (b + 1)],
                    pt[:],
                )

            def pv_unit(j, h, b):
                c, tt = h >> 1, h & 1
                yps = psum.tile([P, DH + 1], F32, tag="y", bufs=2,
                                name=f"y_{j}_{h}_{b}")
                chunks = list(range(4 * j)) + [4 * j + r for r in range(b + 1)]
                for idx, ii in enumerate(chunks):
                    sep, half = se_map[(j, h, ii)]
                    st = (sep[:, half, 128 * b : 128 * (b + 1)]
                          if half is not None
                          else sep[:, 128 * b : 128 * (b + 1)])
                    nc.tensor.matmul(
                        yps[:], st, v_sb[:, h, ii, :],
                        start=(idx == 0), stop=(idx == len(chunks) - 1),
                    )
                rec = small.tile([P, 1], F32, tag="rec", name=f"rec_{j}_{h}_{b}")
                nc.vector.reciprocal(rec[:], yps[:, DH : DH + 1])
                if tt == 0:
                    yf2 = small.tile([P, 2, DH], BF, tag="yf", name=f"yf_{j}_{c}_{b}")
                    pend_yf[(j, c, b)] = yf2
                else:
                    yf2 = pend_yf.pop((j, c, b))
                nc.vector.tensor_scalar_mul(yf2[:, tt, :], yps[:, 0:DH], rec[:])
                if tt == 1:
                    pend_t.append((j, c, b, yf2))
                    if len(pend_t) > 1:
                        flush_transpose()

            # --- schedule ---
            # Filler distribution targets per-slice PE ~= per-slice ACT (exp
            # grows linearly with j, so out-projections are deferred to the
            # later, exp-heavy slices).
            def proj_units(j):
                u = []
                for c in range(2):
                    u.append(lambda c=c: proj_qk_unit(j, wq_sb, qT_sb, c))
                    u.append(lambda c=c: proj_qk_unit(j, wk_sb, kT_sb, c))
                for i in range(4):
                    u.append(lambda i=i: proj_v_unit(j, i))
                return u

            def outproj_units(j):
                return [
                    lambda tb=tb, eb=eb: outproj_unit(j, tb, eb)
                    for tb in range(4)
                    for eb in range(2)
                ]

            p0 = proj_units(0)
            p0[0]()  # q c0
            p0[2]()  # k c0
            p0[1]()  # q c1
            p0[3]()  # k c1
            p3 = proj_units(3)
            slice_fillers = {
                0: [p0[4], p0[5], p0[6], p0[7]] + proj_units(1),
                1: proj_units(2),
                2: [p3[0], p3[2], p3[4], p3[5]] + outproj_units(0),
                3: [p3[1], p3[3], p3[6], p3[7]] + outproj_units(1) + outproj_units(2)[:4],
            }
            tail_reserve = outproj_units(2)[4:]

            pv_queue = []
            for j in range(TJ):
                fillers = slice_fillers[j]
                if j + 2 < TJ:
                    load_x(j + 2)

                tick = [0]

                def pump():
                    # Alternate filler/PV so deadline-ordered fillers (q/k
                    # c1 before head 2 of slice 0; projections feeding the
                    # next slice) drain even while PV units are queued.
                    tick[0] ^= 1
                    if fillers and (tick[0] or not pv_queue):
                        fillers.pop(0)()
                    elif pv_queue:
                        jj, h, b = pv_queue.pop(0)
                        pv_unit(jj, h, b)

                while pv_queue and pv_queue[0][0] < j - 1:
                    jj, h, b = pv_queue.pop(0)
                    pv_unit(jj, h, b)
                for c in range(2):
                    if c == 1:
                        # Old-slice c0-head PV units must emit before this
                        # slice's c1 S-tiles rotate into their se slots (40
                        # slot rotation; reads emitted after the overwriting
                        # write would see the new data). c1-head units are
                        # safe until the next slice starts.
                        keep = []
                        while pv_queue and pv_queue[0][0] < j:
                            jj, h, b = pv_queue.pop(0)
                            if h < 2:
                                pv_unit(jj, h, b)
                            else:
                                keep.append((jj, h, b))
                        pv_queue[0:0] = keep
                    for tt in range(2):
                        for pp in range(2 * j):
                            s_pair(j, 2 * c + tt, pp)
                            pump()
                    for r in range(4):
                        s_diag2(j, c, r)
                        pump()
                    pv_queue.extend((j, 2 * c + tt, b) for b in range(4) for tt in range(2))

                if j < TJ - 1:
                    while fillers:
                        pump()
                else:
                    # Tail: weave the last slice's output projection in as
                    # each tq-block's final transpose lands.
                    fillers.extend(tail_reserve)
                    while pv_queue:
                        jj, h, b = pv_queue.pop(0)
                        pv_unit(jj, h, b)
                        if fillers:
                            fillers.pop(0)()
                        if jj == TJ - 1 and h == NH - 1 and b > 0:
                            while pend_t and pend_t[0][2] < b:
                                flush_transpose()
                            outproj_unit(j, b - 1, 0)
                            outproj_unit(j, b - 1, 1)
                    while pend_t:
                        flush_transpose()
                    while fillers:
                        fillers.pop(0)()
                    outproj_unit(j, 3, 0)
                    outproj_unit(j, 3, 1)

            if dump_debug:
                nc.sync.dma_start(dbg_q, qT_sb[:])
                nc.sync.dma_start(dbg_k, kT_sb[:])
                nc.sync.dma_start(dbg_v, v_sb[:])
                nc.sync.dma_start(dbg_y, yT_sb[:])
    # move_matmul_waits_to_ldweights moves a matmul's sem waits to "the most
    # recent Ldweights". fp32r matmuls have no Ldweights of their own, so in
    # this mixed fp8/fp32r kernel the pass relocates their RAW waits onto an
    # unrelated earlier fp8 Ldweights, dropping the ordering (observed: the
    # first scores matmul reading q/k before the projection drains). Disable.
    nc.move_matmul_waits_to_ldweights = lambda: None
    nc.compile()
    return nc


def make_mask() -> np.ndarray:
    q = np.arange(512)[None, None, :]
    p = np.arange(P)[:, None, None]
    r = np.arange(4)[None, :, None]
    return (q >= 128 * r + p).astype(ml_dtypes.bfloat16)


def _hilo(a: np.ndarray):
    hi = a.astype(ml_dtypes.float8_e4m3)
    lo = (a - hi.astype(np.float32)).astype(ml_dtypes.float8_e4m3)
    return hi, lo


def _chunked_hilo(a: np.ndarray, width: int, pre_scale: float = 1.0):
    """[D, width] f32 -> [P, DC, 2, width] fp8 (hi/lo interleaved)."""
    hi, lo = _hilo(a * pre_scale)
    s = np.stack([hi.reshape(DC, P, width), lo.reshape(DC, P, width)], axis=2)
    return np.ascontiguousarray(s.transpose(1, 0, 2, 3))


def shard_inputs(x, Wqkv, Wout):
    mask = make_mask()
    identity = np.eye(P).astype(ml_dtypes.bfloat16)
    in_maps = []
    for c in range(N_CORES):
        b, g = c // 4, c % 4
        sl = slice(F * g, F * (g + 1))
        xT = np.ascontiguousarray(x[b].T)  # [D, T]
        woT = np.ascontiguousarray(Wout[:, sl].T).astype(ml_dtypes.bfloat16)  # [F, D]
        in_maps.append(
            {
                "xd": _chunked_hilo(xT, T),
                "wq": _chunked_hilo(np.ascontiguousarray(Wqkv[sl, :].T), F, 16.0),
                "wk": _chunked_hilo(np.ascontiguousarray(Wqkv[D:][sl, :].T), F, 16.0),
                "wv": _chunked_hilo(np.ascontiguousarray(Wqkv[2 * D:][sl, :].T), F, 16.0),
                "wo": np.ascontiguousarray(woT.reshape(2, P, D).transpose(1, 0, 2)),
                "mask": mask,
                "ident": identity,
            }
        )
    return in_maps


_NC_CACHE = None


def kernel(x, Wqkv, Wout):
    global _NC_CACHE
    x = np.asarray(x, dtype=np.float32)
    Wqkv = np.asarray(Wqkv, dtype=np.float32)
    Wout = np.asarray(Wout, dtype=np.float32)
    if _NC_CACHE is None:
        _NC_CACHE = build()
    nc = _NC_CACHE
    in_maps = shard_inputs(x, Wqkv, Wout)
    res = run_bass_kernel_spmd(nc, in_maps, core_ids=list(range(N_CORES)))
    outs = [res.results[c]["out"].astype(np.float32) for c in range(N_CORES)]
    return np.stack(
        [outs[0] + outs[1] + outs[2] + outs[3], outs[4] + outs[5] + outs[6] + outs[7]]
    )


# revision 8
# speedup vs baseline: 1.0742x; 1.0015x over previous
"""Multi-head causal self-attention (B=2, T=2048, D=1024, H=16, Dh=64) on 8 TRN2 cores.

Sharding: data-parallel over batch (2 groups of 4 cores), tensor-parallel over
heads within a group (4 heads/core). Host sums the 4 partial outputs per batch.

v2 design (per core), driven by the TimelineSim cost model (matmul cost =
out-free-size x cycles/row; fp8e4+DoubleRow = 0.5, bf16 = 1.0 at any N):
  - QKV projections run in fp8 with an exact-to-~0.2% residual scheme:
    x is shipped as x8 + xr8 (fp8 value + fp8 residual), W as hi + lo fp8.
    Per d-chunk: one DoubleRow matmul computes (Whi+Wlo)@x8 (hi/lo stacked in
    the two k-tiles); per chunk-pair one DoubleRow matmul computes Whi@xr8.
    => 0.75 cycles/row instead of 1.0, with bf16-class accuracy.
  - Scores S^T = K^T Q per 128-chunk in bf16 (charge = S elements / 128).
    exp on ACT in chunk-pairs ([128,1024] per instruction) to amortize the
    ~370ns per-instruction SBUF-access overhead; diagonal chunks get
    column-trimmed singles + a Pool-engine triangular mask multiply.
  - PV is FLIPPED: out y[tq=128, dh+1] with se as stationary and v (+ones
    column) as moving => charge 65/chunk instead of 512 (bf16 has no N>=256
    requirement). Accumulation is per (head, tq-block) over tk chunks; causal
    skips diagonal chunks r > b. Softmax sums come from the ones column; DVE
    reciprocal + per-partition tensor_scalar multiply normalizes and casts to
    bf16; a PE transpose (vs shipped identity) restores yT for the output
    projection; DVE drains the transposed tile to SBUF.
  - Output projection in bf16; output DMA'd as bf16 and upcast on host.
  - Engine budget: PE ~93us, ACT (exp) ~79us, DVE (drains+normalize) ~70us,
    Pool (masks) ~25us. PE instructions are emitted manually interleaved
    (S-blocks / PV units / projection units) so PSUM WAR chains never
    head-of-line block the PE queue.  GPSIMD cannot touch PSUM, so all PSUM
    drains are on DVE/ACT.
"""
import sys

import numpy as np
import ml_dtypes

for _p in ("/opt/trn_rl_repo", "/root/.axon_site/_ro/trn_rl_repo"):
    if _p not in sys.path:
        try:
            import concourse  # noqa: F401
            break
        except ImportError:
            sys.path.append(_p)

import concourse.bass as bass  # noqa: E402
import concourse.tile as tile  # noqa: E402
from concourse import bacc, mybir  # noqa: E402
from concourse.bass_utils import run_bass_kernel_spmd  # noqa: E402

P = 128
T = 2048
D = 1024
NH = 4          # heads per core
DH = 64
F = NH * DH     # per-core head features (256)
DC = D // P     # 8 contraction chunks
TJ = T // 512   # 4 tq slices
TC = T // P     # 16 tk chunks
N_CORES = 8
F8 = mybir.dt.float8e4
BF = mybir.dt.bfloat16
F32 = mybir.dt.float32
FR = mybir.dt.float32r
DR = mybir.MatmulPerfMode.DoubleRow
AF = mybir.ActivationFunctionType


def build(dump_debug=False):
    nc = bacc.Bacc("TRN2", target_bir_lowering=False, debug=False, num_devices=N_CORES)
    xd = nc.dram_tensor("xd", [P, DC, 2, T], F8, kind="ExternalInput").ap()
    wq = nc.dram_tensor("wq", [P, 2, DC, 2, F // 2], F8, kind="ExternalInput").ap()
    wk = nc.dram_tensor("wk", [P, 2, DC, 2, F // 2], F8, kind="ExternalInput").ap()
    wv = nc.dram_tensor("wv", [P, DC, 2, F], F8, kind="ExternalInput").ap()
    wo = nc.dram_tensor("wo", [P, 2, D], BF, kind="ExternalInput").ap()
    mask = nc.dram_tensor("mask", [P, 4, 512], BF, kind="ExternalInput").ap()
    ident = nc.dram_tensor("ident", [P, P], BF, kind="ExternalInput").ap()
    out = nc.dram_tensor("out", [T, D], BF, kind="ExternalOutput").ap()
    if dump_debug:
        dbg_q = nc.dram_tensor("dbg_q", [P, 2, T], FR, kind="ExternalOutput").ap()
        dbg_k = nc.dram_tensor("dbg_k", [P, 2, T], FR, kind="ExternalOutput").ap()
        dbg_v = nc.dram_tensor("dbg_v", [P, NH, TC, DH + 1], BF, kind="ExternalOutput").ap()
        dbg_y = nc.dram_tensor("dbg_y", [P, 2, T], BF, kind="ExternalOutput").ap()
        dbg_se = nc.dram_tensor("dbg_se", [P, 2, 512], BF, kind="ExternalOutput").ap()
        dbg_se1 = nc.dram_tensor("dbg_se1", [P, 2, 512], BF, kind="ExternalOutput").ap()
        dbg_sp = nc.dram_tensor("dbg_sp", [P, 2, 512], F32, kind="ExternalOutput").ap()

    # Wq/Wk/Wv are host-scaled by 16 so their fp8 residuals don't underflow
    # e4m3's exponent range; q.k then carries 256x, absorbed into the exp
    # scale. v carries 16x, absorbed in the output-projection drain.
    scale = 1.0 / (np.sqrt(DH) * 256.0)
    OSCALE = 1.0 / 16.0

    with tile.TileContext(nc) as tc:
        with (
            tc.tile_pool(name="weights", bufs=1) as wpool,
            tc.tile_pool(name="persist", bufs=1) as persist,
            tc.tile_pool(name="x", bufs=2) as xpool,
            tc.tile_pool(name="sexp", bufs=1) as sepool,
            tc.tile_pool(name="small", bufs=4) as small,
            tc.tile_pool(name="outsb", bufs=4) as opool,
            tc.tile_pool(name="ps", bufs=1, space="PSUM") as psum,
        ):
            wq_sb = wpool.tile([P, 2, DC, 2, F // 2], F8)
            wk_sb = wpool.tile([P, 2, DC, 2, F // 2], F8)
            wv_sb = wpool.tile([P, DC, 2, F], F8)
            wo_sb = wpool.tile([P, 2, D], BF)
            mask_sb = wpool.tile([P, 4, 512], BF)
            id_sb = wpool.tile([P, P], BF)

            # q/k live in fp32r: the scores matmuls then self-load weights
            # (no 50ns-SEQ Ldweights per matmul, unlike 16-bit dtypes).
            qT_sb = persist.tile([P, 2, T], FR)
            kT_sb = persist.tile([P, 2, T], FR)
            v_sb = persist.tile([P, NH, TC, DH + 1], BF)
            yT_sb = persist.tile([P, 2, T], BF)

            x_tiles = {}

            def load_x(j):
                x_sb = xpool.tile([P, DC, 2, 512], F8, tag="x", name=f"x_{j}")
                nc.sync.dma_start(x_sb[:], xd[:, :, :, 512 * j : 512 * (j + 1)])
                x_tiles[j] = x_sb

            # Startup preload: few big DMAs (each dma_start costs ~625ns of
            # exclusive HWDGE on top of the transfer, so per-chunk interleave
            # makes the first projection DMA-latency-paced).
            x0_sb = xpool.tile([P, DC, 2, 512], F8, tag="x", name="x_0")
            x_tiles[0] = x0_sb
            # x8 plane first: projection I1 matmuls need only x8; the xr8
            # residual plane (used by the trailing I2 matmuls) follows wk.
            nc.sync.dma_start(wq_sb[:, 0], wq[:, 0])
            nc.sync.dma_start(x0_sb[:, :, 0:1], xd[:, :, 0:1, 0:512])
            nc.sync.dma_start(wk_sb[:, 0], wk[:, 0])
            nc.sync.dma_start(x0_sb[:, :, 1:2], xd[:, :, 1:2, 0:512])
            nc.sync.dma_start(wq_sb[:, 1], wq[:, 1])
            nc.sync.dma_start(wk_sb[:, 1], wk[:, 1])
            nc.sync.dma_start(wv_sb[:], wv)
            nc.sync.dma_start(mask_sb[:], mask)
            nc.sync.dma_start(id_sb[:], ident)
            load_x(1)
            nc.sync.dma_start(wo_sb[:], wo)

            # ones column of V (softmax sums); gpsimd memset is SBUF-only.
            nc.gpsimd.memset(v_sb[:, :, :, DH : DH + 1], 1.0)

            # --- projection units (fp8 residual DoubleRow) ---
            def proj_qk_unit(j, w_sb, dst, c):
                jsl = slice(512 * j, 512 * (j + 1))
                x_sb = x_tiles[j]
                pt = psum.tile([P, 512], F32, tag="ao", bufs=2, name=f"pqk_{j}_{c}")
                n_i = DC + DC // 2
                k = 0
                for o in range(DC):
                    nc.tensor.matmul(
                        pt[:],
                        w_sb[:, c, o, :, :],
                        x_sb[:, o, 0, None, :].to_broadcast([P, 2, 512]),
                        start=(k == 0), stop=(k == n_i - 1), perf_mode=DR,
                    )
                    k += 1
                for o in range(0, DC, 2):
                    nc.tensor.matmul(
                        pt[:],
                        w_sb[:, c, o : o + 2, 0, :],
                        x_sb[:, o : o + 2, 1, :],
                        start=(k == 0), stop=(k == n_i - 1), perf_mode=DR,
                    )
                    k += 1
                nc.vector.tensor_copy(dst[:, c, jsl], pt[:])

            def proj_v_unit(j, i):
                x_sb = x_tiles[j]
                pt = psum.tile([P, 512], F32, tag="ao", bufs=2, name=f"pv_{j}_{i}")
                n_i = DC + DC // 2
                k = 0
                for o in range(DC):
                    nc.tensor.matmul(
                        pt[:, :F],
                        x_sb[:, o, :, 128 * i : 128 * (i + 1)],
                        wv_sb[:, o, 0, None, :].to_broadcast([P, 2, F]),
                        start=(k == 0), stop=(k == n_i - 1), perf_mode=DR,
                    )
                    k += 1
                for o in range(0, DC, 2):
                    nc.tensor.matmul(
                        pt[:, :F],
                        x_sb[:, o : o + 2, 0, 128 * i : 128 * (i + 1)],
                        wv_sb[:, o : o + 2, 1, :],
                        start=(k == 0), stop=(k == n_i - 1), perf_mode=DR,
                    )
                    k += 1
                nc.vector.tensor_copy(
                    v_sb[:, :, 4 * j + i, 0:DH],
                    pt[:, :F].rearrange("p (h d) -> p h d", h=NH),
                )

            def outproj_unit(j, tb, eb):
                while pend_t:
                    flush_transpose()
                pt = psum.tile([P, 512], F32, tag="ao", bufs=2, name=f"o_{j}_{tb}_{eb}")
                for g in range(2):
                    nc.tensor.matmul(
                        pt[:],
                        yT_sb[:, g, 128 * (4 * j + tb) : 128 * (4 * j + tb + 1)],
                        wo_sb[:, g, 512 * eb : 512 * (eb + 1)],
                        start=(g == 0), stop=(g == 1),
                    )
                osb = opool.tile([P, 512], BF, tag="osb", name=f"osb_{j}_{tb}_{eb}")
                rows = slice(128 * (4 * j + tb), 128 * (4 * j + tb + 1))
                if j == TJ - 1 and tb == 3:
                    for hf in range(2):
                        cs = slice(256 * hf, 256 * (hf + 1))
                        nc.scalar.mul(osb[:, cs], pt[:, cs], OSCALE)
                        nc.sync.dma_start(
                            out[rows, 512 * eb + 256 * hf : 512 * eb + 256 * (hf + 1)],
                            osb[:, cs],
                        )
                elif j == TJ - 1:
                    nc.scalar.mul(osb[:], pt[:], OSCALE)
                    nc.sync.dma_start(out[rows, 512 * eb : 512 * (eb + 1)], osb[:])
                else:
                    nc.vector.tensor_scalar_mul(osb[:], pt[:], OSCALE)
                    nc.sync.dma_start(out[rows, 512 * eb : 512 * (eb + 1)], osb[:])

            # --- attention emitters ---
            se_map = {}

            def s_pair(j, h, pp):
                c, tt = h >> 1, h & 1
                hp = DH * tt
                jsl = slice(512 * j, 512 * (j + 1))
                sps = psum.tile([P, 2, 512], F32, tag="sp", bufs=2,
                                name=f"sp_{j}_{h}_{pp}")
                sep = sepool.tile([P, 2, 512], BF, tag="sep", bufs=40,
                                  name=f"sep_{j}_{h}_{pp}")
                for q in range(2):
                    ii = 2 * pp + q
                    nc.tensor.matmul(
                        sps[:, q, :],
                        kT_sb[hp : hp + DH, c, 128 * ii : 128 * (ii + 1)],
                        qT_sb[hp : hp + DH, c, jsl],
                        start=True, stop=True,
                    )
                    se_map[(j, h, ii)] = (sep, q)
                nc.scalar.activation(sep[:], sps[:], AF.Exp, scale=scale)

            def s_diag2(j, c, r):
                # Both heads of c-pair share the diagonal width, so their two
                # S chunks go in one "sp" pair slot and get ONE exp + ONE
                # (broadcast-mask) multiply.
                ii = 4 * j + r
                col0 = 128 * r
                sdp = psum.tile([P, 2, 512], F32, tag="sp", bufs=2,
                                name=f"sd_{j}_{c}_{r}")
                sed = sepool.tile([P, 2, 512], BF, tag="sep", bufs=40,
                                  name=f"sed_{j}_{c}_{r}")
                # fp32r matmuls with N<256 cost 4 cycles/row; widen r=3 to
                # N=256 (extra columns land in never-read se space).
                mcol0 = min(col0, 512 - 256)
                for tt in range(2):
                    hp = DH * tt
                    nc.tensor.matmul(
                        sdp[:, tt, mcol0:],
                        kT_sb[hp : hp + DH, c, 128 * ii : 128 * (ii + 1)],
                        qT_sb[hp : hp + DH, c, 512 * j + mcol0 : 512 * (j + 1)],
                        start=True, stop=True,
                    )
                    se_map[(j, 2 * c + tt, ii)] = (sed, tt)
                if dump_debug and (j, c, r) == (0, 0, 0):
                    spc = small.tile([P, 2, 512], F32, tag="dbgsp", name="spc")
                    nc.vector.tensor_copy(spc[:], sdp[:])
                    nc.sync.dma_start(dbg_sp, spc[:])
                nc.scalar.activation(
                    sed[:, :, col0:], sdp[:, :, col0:], AF.Exp, scale=scale
                )
                nc.gpsimd.tensor_mul(
                    sed[:, :, col0 : col0 + 128],
                    sed[:, :, col0 : col0 + 128],
                    mask_sb[:, r, None, col0 : col0 + 128].to_broadcast([P, 2, 128]),
                )
                if dump_debug and (j, c, r) == (0, 0, 0):
                    nc.sync.dma_start(dbg_se, sed[:])
                if dump_debug and (j, c, r) == (0, 0, 1):
                    nc.sync.dma_start(dbg_se1, sed[:])

            pend_t = []
            pend_yf = {}

            def flush_transpose():
                # One transpose per head-PAIR: yf2 holds both heads' columns,
                # the transpose emits the full 128-partition yT block.
                jj, c, b, yf2 = pend_t.pop(0)
                pt = psum.tile([P, P], BF, tag="ao", bufs=2, name=f"t_{jj}_{c}_{b}")
                nc.tensor.transpose(pt[:], yf2[:], id_sb[:])
                nc.vector.tensor_copy(
                    yT_sb[:, c, 512 * jj + 128 * b : 512 * jj + 128 * (b + 1)],
                    pt[:],
                )

            def pv_unit(j, h, b):
                c, tt = h >> 1, h & 1
                yps = psum.tile([P, DH + 1], F32, tag="y", bufs=2,
                                name=f"y_{j}_{h}_{b}")
                chunks = list(range(4 * j)) + [4 * j + r for r in range(b + 1)]
                for idx, ii in enumerate(chunks):
                    sep, half = se_map[(j, h, ii)]
                    st = (sep[:, half, 128 * b : 128 * (b + 1)]
                          if half is not None
                          else sep[:, 128 * b : 128 * (b + 1)])
                    nc.tensor.matmul(
                        yps[:], st, v_sb[:, h, ii, :],
                        start=(idx == 0), stop=(idx == len(chunks) - 1),
                    )
                rec = small.tile([P, 1], F32, tag="rec", name=f"rec_{j}_{h}_{b}")
                nc.vector.reciprocal(rec[:], yps[:, DH : DH + 1])
                if tt == 0:
                    yf2 = small.tile([P, 2, DH], BF, tag="yf", name=f"yf_{j}_{c}_{b}")
                    pend_yf[(j, c, b)] = yf2
                else:
                    yf2 = pend_yf.pop((j, c, b))
                nc.vector.tensor_scalar_mul(yf2[:, tt, :], yps[:, 0:DH], rec[:])
                if tt == 1:
                    pend_t.append((j, c, b, yf2))
                    if len(pend_t) > 1:
                        flush_transpose()

            # --- schedule ---
            # Filler distribution targets per-slice PE ~= per-slice ACT (exp
            # grows linearly with j, so out-projections are deferred to the
            # later, exp-heavy slices).
            def proj_units(j):
                u = []
                for c in range(2):
                    u.append(lambda c=c: proj_qk_unit(j, wq_sb, qT_sb, c))
                    u.append(lambda c=c: proj_qk_unit(j, wk_sb, kT_sb, c))
                for i in range(4):
                    u.append(lambda i=i: proj_v_unit(j, i))
                return u

            def outproj_units(j):
                return [
                    lambda tb=tb, eb=eb: outproj_unit(j, tb, eb)
                    for tb in range(4)
                    for eb in range(2)
                ]

            p0 = proj_units(0)
            p0[0]()  # q c0
            p0[2]()  # k c0
            p0[1]()  # q c1
            p0[3]()  # k c1
            p3 = proj_units(3)
            slice_fillers = {
                0: [p0[4], p0[5], p0[6], p0[7]] + proj_units(1),
                1: proj_units(2),
                2: [p3[0], p3[2], p3[4], p3[5]] + outproj_units(0),
                3: [p3[1], p3[3], p3[6], p3[7]] + outproj_units(1) + outproj_units(2)[:4],
            }
            tail_reserve = outproj_units(2)[4:]

            pv_queue = []
            for j in range(TJ):
                fillers = slice_fillers[j]
                if j + 2 < TJ:
                    load_x(j + 2)

                tick = [0]

                def pump():
                    # Alternate filler/PV so deadline-ordered fillers (q/k
                    # c1 before head 2 of slice 0; projections feeding the
                    # next slice) drain even while PV units are queued.
                    tick[0] ^= 1
                    if fillers and (tick[0] or not pv_queue):
                        fillers.pop(0)()
                    elif pv_queue:
                        jj, h, b = pv_queue.pop(0)
                        pv_unit(jj, h, b)

                while pv_queue and pv_queue[0][0] < j - 1:
                    jj, h, b = pv_queue.pop(0)
                    pv_unit(jj, h, b)
                for c in range(2):
                    if c == 1:
                        # Old-slice c0-head PV units must emit before this
                        # slice's c1 S-tiles rotate into their se slots (40
                        # slot rotation; reads emitted after the overwriting
                        # write would see the new data). c1-head units are
                        # safe until the next slice starts.
                        keep = []
                        while pv_queue and pv_queue[0][0] < j:
                            jj, h, b = pv_queue.pop(0)
                            if h < 2:
                                pv_unit(jj, h, b)
                            else:
                                keep.append((jj, h, b))
                        pv_queue[0:0] = keep
                    for tt in range(2):
                        for pp in range(2 * j):
                            s_pair(j, 2 * c + tt, pp)
                            pump()
                    for r in range(4):
                        s_diag2(j, c, r)
                        pump()
                    pv_queue.extend((j, 2 * c + tt, b) for b in range(4) for tt in range(2))

                if j < TJ - 1:
                    while fillers:
                        pump()
                else:
                    # Tail: weave the last slice's output projection in as
                    # each tq-block's final transpose lands.
                    fillers.extend(tail_reserve)
                    while pv_queue:
                        jj, h, b = pv_queue.pop(0)
                        pv_unit(jj, h, b)
                        if fillers:
                            fillers.pop(0)()
                        if jj == TJ - 1 and h == NH - 1 and b > 0:
                            while pend_t and pend_t[0][2] < b:
                                flush_transpose()
                            outproj_unit(j, b - 1, 0)
                            outproj_unit(j, b - 1, 1)
                    while pend_t:
                        flush_transpose()
                    while fillers:
                        fillers.pop(0)()
                    outproj_unit(j, 3, 0)
                    outproj_unit(j, 3, 1)

            if dump_debug:
                nc.sync.dma_start(dbg_q, qT_sb[:])
                nc.sync.dma_start(dbg_k, kT_sb[:])
                nc.sync.dma_start(dbg_v, v_sb[:])
                nc.sync.dma_start(dbg_y, yT_sb[:])
    # move_matmul_waits_to_ldweights moves a matmul's sem waits to "the most
    # recent Ldweights". fp32r matmuls have no Ldweights of their own, so in
    # this mixed fp8/fp32r kernel the pass relocates their RAW waits onto an
    # unrelated earlier fp8 Ldweights, dropping the ordering (observed: the
    # first scores matmul reading q/k before the projection drains). Disable.
    nc.move_matmul_waits_to_ldweights = lambda: None
    nc.compile()
    return nc


def make_mask() -> np.ndarray:
    q = np.arange(512)[None, None, :]
    p = np.arange(P)[:, None, None]
    r = np.arange(4)[None, :, None]
    return (q >= 128 * r + p).astype(ml_dtypes.bfloat16)


def _hilo(a: np.ndarray):
    hi = a.astype(ml_dtypes.float8_e4m3)
    lo = (a - hi.astype(np.float32)).astype(ml_dtypes.float8_e4m3)
    return hi, lo


def _chunked_hilo(a: np.ndarray, width: int, pre_scale: float = 1.0):
    """[D, width] f32 -> [P, DC, 2, width] fp8 (hi/lo interleaved)."""
    hi, lo = _hilo(a * pre_scale)
    s = np.stack([hi.reshape(DC, P, width), lo.reshape(DC, P, width)], axis=2)
    return np.ascontiguousarray(s.transpose(1, 0, 2, 3))


def _chunked_hilo_cm(a: np.ndarray, pre_scale: float = 1.0):
    """[D, F] f32 -> [P, 2(c), DC, 2(plane), 128] fp8 (c-major)."""
    base = _chunked_hilo(a, F, pre_scale)  # [P, DC, 2, F]
    r = base.reshape(P, DC, 2, 2, F // 2)
    return np.ascontiguousarray(r.transpose(0, 3, 1, 2, 4))


def shard_inputs(x, Wqkv, Wout):
    mask = make_mask()
    identity = np.eye(P).astype(ml_dtypes.bfloat16)
    in_maps = []
    for c in range(N_CORES):
        b, g = c // 4, c % 4
        sl = slice(F * g, F * (g + 1))
        xT = np.ascontiguousarray(x[b].T)  # [D, T]
        woT = np.ascontiguousarray(Wout[:, sl].T).astype(ml_dtypes.bfloat16)  # [F, D]
        in_maps.append(
            {
                "xd": _chunked_hilo(xT, T),
                "wq": _chunked_hilo_cm(np.ascontiguousarray(Wqkv[sl, :].T), 16.0),
                "wk": _chunked_hilo_cm(np.ascontiguousarray(Wqkv[D:][sl, :].T), 16.0),
                "wv": _chunked_hilo(np.ascontiguousarray(Wqkv[2 * D:][sl, :].T), F, 16.0),
                "wo": np.ascontiguousarray(woT.reshape(2, P, D).transpose(1, 0, 2)),
                "mask": mask,
                "ident": identity,
            }
        )
    return in_maps


_NC_CACHE = None


def kernel(x, Wqkv, Wout):
    global _NC_CACHE
    x = np.asarray(x, dtype=np.float32)
    Wqkv = np.asarray(Wqkv, dtype=np.float32)
    Wout = np.asarray(Wout, dtype=np.float32)
    if _NC_CACHE is None:
        _NC_CACHE = build()
    nc = _NC_CACHE
    in_maps = shard_inputs(x, Wqkv, Wout)
    res = run_bass_kernel_spmd(nc, in_maps, core_ids=list(range(N_CORES)))
    outs = [res.results[c]["out"].astype(np.float32) for c in range(N_CORES)]
    return np.stack(
        [outs[0] + outs[1] + outs[2] + outs[3], outs[4] + outs[5] + outs[6] + outs[7]]
    )
